# revision 3
# baseline (speedup 1.0000x reference)
"""CustomLSTM cell on 8 TRN2 NeuronCores — fp8/bf16 hybrid matmul.

Data-parallel over batch (4096 rows/core). The 4 gate projections run at
mixed precision chosen so the total error stays ~1.5e-2 (< 2e-2 gate):
  - i, f gates: full-K fp8e4m3 DoubleRow matmuls (2x PE rate)
  - o gate:     fp8 DoubleRow on K 0:256, bf16 on K 256:1024
  - g (tanh) gate: full bf16 (its error coefficient is 3.6x the others)
All W variants are premultiplied by 16 on the host (fp8 subnormal
avoidance); the activation instructions fold the 1/16 dequant into their
scale operand. PSUM chains mix fp8/bf16 matmuls at a consistent scale.

PE work: 9/16 of the GEMM at 2x rate -> ~157us/core vs 218us all-bf16.

Self-contained: shapes/sharding hardcoded for
input [32768, 1024], cell_state [32768, 512], W* [1024, 512].
"""

import os

import numpy as np
import ml_dtypes

import bass_rust
import concourse.bass as bass
import concourse.mybir as mybir
import concourse.tile as tile
from concourse.bass_utils import run_bass_kernel_spmd

N_CORES = 8
B = 32768
D = 1024
H = 512
P = 128
B_LOC = B // N_CORES        # 4096 rows per core
KO = D // P                 # 8 k-subtiles of 128
KS = KO // 2                # 4 DoubleRow k-steps of 256
NW = 4 * H                  # 2048 concatenated gate dim
NG = 4
BG_ROWS = 512               # batch rows per x slab
BG = B_LOC // BG_ROWS       # 8 slabs per core
BT_PER_BG = BG_ROWS // P    # 4 batch tiles per slab

WSCALE = 16.0               # host premultiplies all W; undone in activation

F8 = mybir.dt.float8e4
BF16 = mybir.dt.bfloat16
F32 = mybir.dt.float32
NPF8 = ml_dtypes.float8_e4m3
NPBF = ml_dtypes.bfloat16

LAST_RESULTS = None
_CACHED = {}


def _split_multi_waits(nc):
    """Legalize for a walrus build that accepts one sync-wait per instruction."""
    n = 0
    for f in nc.m.functions:
        for blk in f.blocks:
            insts = blk.instructions
            if not any(
                i.sync_info is not None and len(i.sync_info.on_wait) > 1
                for i in insts
            ):
                continue
            out = []
            for inst in insts:
                si = inst.sync_info
                if si is not None and len(si.on_wait) > 1:
                    waits = list(si.on_wait)
                    for w in waits[:-1]:
                        nop = mybir.InstNoOp(name=f"waitsplit_{n}", ins=[], outs=[])
                        n += 1
                        nop.engine = inst.engine
                        nop.sync_info = bass_rust.SyncInfo(on_wait=[w], on_update=[])
                        out.append(nop)
                    inst.sync_info = bass_rust.SyncInfo(
                        on_wait=[waits[-1]], on_update=list(si.on_update)
                    )
                out.append(inst)
            blk.instructions = out


class _FastTailTileContext(tile.TileContext):
    """Drop both tail all-engine barriers.

    The stock tail is [drain+waits][barrier][sem/queue reset][barrier].  The
    drain's sem waits already cover completion of every instruction and DMA,
    so by the time the gpsimd-side reset runs nothing is in flight that could
    observe the cleared semaphores; NRT waits for each engine stream to halt
    independently.  Saves ~8-10us of EVSEM barrier ring.
    """

    def _drain_and_barrier(self, tick_clock, wait_clock):
        from concourse.vector_clock import ScopedClock

        drain_inst = self.nc.sync.drain()
        wait_clock.add_sem_waits(
            drain_inst.ins, ScopedClock({None: tick_clock.global_clock})
        )
        # Chain the gpsimd-side reset directly behind the drain (gpsimd has
        # been idle since the warmup memset; without this it would clear
        # live semaphores immediately).
        tail_sem = self.nc.alloc_semaphore("fast_tail_sem")
        drain_inst.then_inc(tail_sem)
        self.nc.gpsimd.wait_ge(tail_sem, 1)
        assert self.sems is not None
        popped = self.nc._tile_sem_poison_stack.pop()
        assert popped is self._sem_poison
        self.nc.clear_and_free_semaphores(list(self.sems.allocated().values()))


def _build(with_bias):
    nc = bass.Bass()
    AF = mybir.ActivationFunctionType
    ts = bass.ts
    DR = mybir.MatmulPerfMode.DoubleRow
    SCL = 1.0 / WSCALE

    x8t = nc.dram_tensor("x8t", [BG, P, KO, BG_ROWS], F8, kind="ExternalInput")
    xbt = nc.dram_tensor("xbt", [BG, P, KO, BG_ROWS], BF16, kind="ExternalInput")
    w8 = nc.dram_tensor("w8", [P, KO, 2 * H], F8, kind="ExternalInput")   # i|f
    w8o = nc.dram_tensor("w8o", [P, 4, H], F8, kind="ExternalInput")      # o k<512
    wb = nc.dram_tensor("wb", [P, KO, 2 * H], BF16, kind="ExternalInput")  # g|o
    cell = nc.dram_tensor("cell", [B_LOC, H], BF16, kind="ExternalInput")
    if with_bias:
        bias = nc.dram_tensor("bias", [P, NW], F32, kind="ExternalInput")
    h_out = nc.dram_tensor("h_out", [B_LOC, H], BF16, kind="ExternalOutput")
    c_out = nc.dram_tensor("c_out", [B_LOC, H], BF16, kind="ExternalOutput")

    with _FastTailTileContext(nc) as tc:
        with (
            tc.tile_pool(name="wpool", bufs=1) as wpool,
            tc.tile_pool(name="x8pool", bufs=3) as x8pool,
            tc.tile_pool(name="xbpool", bufs=3) as xbpool,
            tc.tile_pool(name="cpool", bufs=4) as cpool,
            tc.tile_pool(name="gpool", bufs=3) as gpool,
            tc.tile_pool(name="ppool", bufs=8, space="PSUM") as ppool,
        ):
            # PE warmup first: runs while the startup DMA triggers issue, so
            # the HAM clock gate opens before real data lands.
            wz = wpool.tile([P, P], F8, tag="wz", name="wz")
            nc.gpsimd.memset(wz[:], 0.0)
            warm_ps = ppool.tile([P, P], F32, tag="ps", name="warm_ps")
            for _ in range(16):
                nc.tensor.matmul(warm_ps[:], wz[:], wz[:], start=True, stop=True)

            bias_t = None
            if with_bias:
                bias_t = wpool.tile([P, NW], F32, tag="bias_t", name="bias_t")
                nc.sync.dma_start(bias_t[:], bias[:])

            # Resident weight tiles + slab-0 x, DMA'd in consumption order:
            # fp8 stream first (slab 0 runs its fp8 phase first), then the
            # bf16 stream, with the first pair's cell tiles slotted where the
            # epilogues need them.
            w8t = wpool.tile([P, KO, 2 * H], F8, tag="w8t", name="w8t")
            w8ot = wpool.tile([P, 4, H], F8, tag="w8ot", name="w8ot")
            wbt = wpool.tile([P, KO, 2 * H], BF16, tag="wbt", name="wbt")
            x8s0 = wpool.tile([P, KO, BG_ROWS], F8, tag="x8s0", name="x8s0")
            xbs0 = wpool.tile([P, KO, BG_ROWS], BF16, tag="xbs0", name="xbs0")
            cts0 = [
                cpool.tile([P, H], BF16, tag="ct", name=f"ct0_{j}")
                for j in range(BT_PER_BG)
            ]
            for k in range(KO):
                nc.sync.dma_start(x8s0[:, k, :], x8t[0, :, k, :])
                nc.sync.dma_start(w8t[:, k, :], w8[:, k, :])
                if k == 1:
                    # Priority gap: first two k-chunks gate the first matmul.
                    for dd in range(2):
                        scr = wpool.tile([P, 16], F8, tag=f"scr{dd}", name=f"scr{dd}")
                        nc.sync.dma_start(scr[:], x8t[0, :, 0, :16])
            nc.sync.dma_start(w8ot[:], w8o[:])
            for k in range(KO):
                nc.sync.dma_start(xbs0[:, k, :], xbt[0, :, k, :])
                nc.sync.dma_start(wbt[:, k, :H], wb[:, k, :H])  # g columns
            for j in (0, 1):
                nc.sync.dma_start(cts0[j][:], cell[j * P : (j + 1) * P, :])
            for k in range(4, KO):
                nc.sync.dma_start(wbt[:, k, H:], wb[:, k, H:])  # o columns
            for j in (2, 3):
                nc.sync.dma_start(cts0[j][:], cell[j * P : (j + 1) * P, :])

            def mm_btile(ps, x8s, xbs, j):
                """i,f full fp8; o fp8 K<512 + bf16 k4..7; g full bf16."""
                for ks in range(KS):
                    l8 = x8s[:, 2 * ks : 2 * ks + 2, ts(j, P)]
                    for gi in (0, 1):  # i then f
                        for c in range(2):
                            nc.tensor.matmul(
                                ps[gi][:, ts(c, 256)],
                                l8,
                                w8t[:, 2 * ks : 2 * ks + 2,
                                    gi * H + c * 256 : gi * H + (c + 1) * 256],
                                start=(ks == 0 and c == 0),
                                stop=(ks == KS - 1),
                                perf_mode=DR,
                                skip_group_check=True,
                            )
                    if ks < 2:
                        for c in range(2):  # o's fp8 steps ride i/f's lhsT
                            nc.tensor.matmul(
                                ps[3][:, ts(c, 256)],
                                l8,
                                w8ot[:, 2 * ks : 2 * ks + 2,
                                     c * 256 : (c + 1) * 256],
                                start=(ks == 0 and c == 0),
                                stop=False,
                                perf_mode=DR,
                                skip_group_check=True,
                            )
                for k in range(KO):
                    lb = xbs[:, k, ts(j, P)]
                    nc.tensor.matmul(
                        ps[2], lb, wbt[:, k, :H], start=(k == 0), stop=(k == KO - 1)
                    )
                    if k >= 4:
                        nc.tensor.matmul(
                            ps[3], lb, wbt[:, k, H:],
                            start=False, stop=(k == KO - 1),
                            skip_group_check=True,
                        )

            def epilogue(ps, ct, rows, uid, splits=1):
                # psum order: 0=i 1=f 2=g 3=o. activation scale undoes the
                # host-side 16x W premultiply.
                if with_bias:
                    zs = []
                    for nn in range(NG):
                        z = gpool.tile([P, H], F32, tag=f"z{nn}", name=f"z{nn}_{uid}")
                        nc.vector.tensor_add(z[:], ps[nn], bias_t[:, ts(nn, H)])
                        zs.append(z)
                else:
                    zs = ps
                w_ = H // splits
                for q in range(splits):
                    cs = slice(q * w_, (q + 1) * w_)
                    # all four activations first: a gate's PSUM bank frees at
                    # its activation read, so front-loading them unblocks the
                    # bank rotation for btile N+2 ~2us earlier.
                    i_t = gpool.tile([P, w_], F32, tag="i_t", name=f"i_{uid}_{q}")
                    nc.scalar.activation(i_t[:], zs[0][:, cs], AF.Sigmoid, scale=SCL)
                    f_t = gpool.tile([P, w_], F32, tag="f_t", name=f"f_{uid}_{q}")
                    nc.scalar.activation(f_t[:], zs[1][:, cs], AF.Sigmoid, scale=SCL)
                    g_t = gpool.tile([P, w_], F32, tag="g_t", name=f"g_{uid}_{q}")
                    nc.scalar.activation(g_t[:], zs[2][:, cs], AF.Tanh, scale=SCL)
                    o_t = gpool.tile([P, w_], F32, tag="o_t", name=f"o_{uid}_{q}")
                    nc.scalar.activation(o_t[:], zs[3][:, cs], AF.Sigmoid, scale=SCL)

                    fc = gpool.tile([P, w_], F32, tag="fc", name=f"fc_{uid}_{q}")
                    nc.vector.tensor_mul(fc[:], f_t[:], ct[:, cs])
                    ig = gpool.tile([P, w_], F32, tag="ig", name=f"ig_{uid}_{q}")
                    nc.vector.tensor_mul(ig[:], i_t[:], g_t[:])
                    cn = gpool.tile([P, w_], BF16, tag="cn", name=f"cn_{uid}_{q}")
                    nc.vector.tensor_add(cn[:], fc[:], ig[:])
                    tn = gpool.tile([P, w_], F32, tag="tn", name=f"tn_{uid}_{q}")
                    nc.scalar.activation(tn[:], cn[:], AF.Tanh)
                    hn = gpool.tile([P, w_], BF16, tag="hn", name=f"hn_{uid}_{q}")
                    nc.vector.tensor_mul(hn[:], o_t[:], tn[:])

                    nc.sync.dma_start(c_out[rows, cs], cn[:])
                    nc.sync.dma_start(h_out[rows, cs], hn[:])
                return cn

            # Slab 0: j-pairs; within a pair run the fp8 phase first, then g,
            # then o's bf16 tail — matching the DMA stream arrival order.
            xslabs = {}

            def prefetch_slab(g):
                # The dma triggers sit on the sequential sync queue behind the
                # preceding epilogue's out-DMA triggers, which wait on that
                # epilogue's results — a natural throttle that keeps prefetch
                # from starving the slab-0 startup stream.
                x8s = x8pool.tile([P, KO, BG_ROWS], F8, tag="x8s", name=f"x8s_{g}")
                nc.sync.dma_start(x8s[:], x8t[g])
                xbs = xbpool.tile([P, KO, BG_ROWS], BF16, tag="xbs", name=f"xbs_{g}")
                nc.sync.dma_start(xbs[:], xbt[g])
                xslabs[g] = (x8s, xbs)

            for jp in (0, 2):
                ps2 = {
                    (j, nn): ppool.tile([P, H], F32, tag="ps", name=f"ps0_{j}_{nn}")
                    for j in (jp, jp + 1)
                    for nn in range(NG)
                }
                # phase 1: fp8 i,f (+ o's fp8 steps, carrying o's chain start)
                for ks in range(KS):
                    for j in (jp, jp + 1):
                        l8 = x8s0[:, 2 * ks : 2 * ks + 2, ts(j, P)]
                        for gi in (0, 1):
                            for c in range(2):
                                nc.tensor.matmul(
                                    ps2[(j, gi)][:, ts(c, 256)],
                                    l8,
                                    w8t[:, 2 * ks : 2 * ks + 2,
                                        gi * H + c * 256 : gi * H + (c + 1) * 256],
                                    start=(ks == 0 and c == 0),
                                    stop=(ks == KS - 1),
                                    perf_mode=DR,
                                    skip_group_check=True,
                                )
                        if ks < 2:
                            for c in range(2):
                                nc.tensor.matmul(
                                    ps2[(j, 3)][:, ts(c, 256)],
                                    l8,
                                    w8ot[:, 2 * ks : 2 * ks + 2,
                                         c * 256 : (c + 1) * 256],
                                    start=(ks == 0 and c == 0),
                                    stop=False,
                                    perf_mode=DR,
                                    skip_group_check=True,
                                )
                # phase 2: gate g, k-major across the pair
                for k in range(KO):
                    for j in (jp, jp + 1):
                        nc.tensor.matmul(
                            ps2[(j, 2)], xbs0[:, k, ts(j, P)], wbt[:, k, :H],
                            start=(k == 0), stop=(k == KO - 1),
                        )
                # phase 3: gate o bf16 tail
                for k in range(4, KO):
                    for j in (jp, jp + 1):
                        nc.tensor.matmul(
                            ps2[(j, 3)], xbs0[:, k, ts(j, P)], wbt[:, k, H:],
                            start=False, stop=(k == KO - 1),
                            skip_group_check=True,
                        )
                for j in (jp, jp + 1):
                    epilogue(
                        [ps2[(j, nn)] for nn in range(NG)],
                        cts0[j],
                        slice(j * P, (j + 1) * P),
                        f"g0_{j}",
                    )
                    if j == 1:
                        prefetch_slab(1)
                    elif j == 3:
                        prefetch_slab(2)

            # Slabs 1..7 against prefetched slabs (next slab requested after
            # this slab's first epilogue — ~3 btiles of lead time).
            for g in range(1, BG):
                x8s, xbs = xslabs[g]
                for j in range(BT_PER_BG):
                    bt = g * BT_PER_BG + j
                    rows = slice(bt * P, (bt + 1) * P)
                    ct = cpool.tile([P, H], BF16, tag="ct", name=f"ct_{bt}")
                    nc.sync.dma_start(ct[:], cell[rows, :])
                    ps = [
                        ppool.tile([P, H], F32, tag="ps", name=f"ps{nn}_{bt}")
                        for nn in range(NG)
                    ]
                    mm_btile(ps, x8s, xbs, j)
                    last_bt = g == BG - 1 and j == BT_PER_BG - 1
                    epilogue(ps, ct, rows, f"g{g}_{j}", splits=2 if last_bt else 1)
                    if j == 0 and g + 1 < BG:
                        prefetch_slab(g + 1)

    _split_multi_waits(nc)
    return nc


def kernel(input, cell_state, Wi, bi, Wf, bf, Wg, bg, Wo, bo):
    global LAST_RESULTS

    x = np.asarray(input, dtype=np.float32)
    cell = np.ascontiguousarray(np.asarray(cell_state, dtype=np.float32).astype(NPBF))
    Wi, Wf, Wg, Wo = (np.asarray(m, dtype=np.float32) for m in (Wi, Wf, Wg, Wo))
    bcat = np.concatenate(
        [np.asarray(v, dtype=np.float32) for v in (bi, bf, bg, bo)]
    )  # [4H] in i,f,g,o order (matches psum order)
    with_bias = bool(np.any(bcat))

    def wlay(Wcat, np_dt):
        # [D, N] -> [p, ko, n], scaled by WSCALE
        n = Wcat.shape[1]
        return np.ascontiguousarray(
            (Wcat * WSCALE).astype(np_dt).reshape(KO, P, n).transpose(1, 0, 2)
        )

    w8_dev = wlay(np.concatenate([Wi, Wf], axis=1), NPF8)          # [P,KO,1024]
    wb_dev = wlay(np.concatenate([Wg, Wo], axis=1), NPBF)          # [P,KO,1024]
    w8o_dev = np.ascontiguousarray(
        (Wo[:512] * WSCALE).astype(NPF8).reshape(4, P, H).transpose(1, 0, 2)
    )  # [P,4,H]

    in_maps = []
    for c in range(N_CORES):
        xc = x[c * B_LOC : (c + 1) * B_LOC]  # [4096, 1024]
        def xlay(np_dt):
            return np.ascontiguousarray(
                xc.astype(np_dt)
                .reshape(BG, BG_ROWS, KO, P)
                .transpose(0, 3, 2, 1)
            )
        m = {
            "x8t": xlay(NPF8),
            "xbt": xlay(NPBF),
            "w8": w8_dev,
            "w8o": w8o_dev,
            "wb": wb_dev,
            "cell": cell[c * B_LOC : (c + 1) * B_LOC],
        }
        if with_bias:
            m["bias"] = np.ascontiguousarray(
                np.broadcast_to(bcat[None, :] * WSCALE, (P, NW)).astype(np.float32)
            )
        in_maps.append(m)

    key = with_bias
    if key not in _CACHED:
        _CACHED[key] = _build(with_bias)
    nc = _CACHED[key]

    trace = os.environ.get("KERNEL_TRACE", "0") == "1"
    res = run_bass_kernel_spmd(nc, in_maps, list(range(N_CORES)), trace=trace)
    LAST_RESULTS = res

    h = np.concatenate(
        [res.results[c]["h_out"].astype(np.float32) for c in range(N_CORES)], axis=0
    )
    c_ = np.concatenate(
        [res.results[c]["c_out"].astype(np.float32) for c in range(N_CORES)], axis=0
    )
    return h, c_


# revision 5
# speedup vs baseline: 1.0559x; 1.0559x over previous
"""CustomLSTM cell on 8 TRN2 NeuronCores — fp8/bf16 hybrid matmul.

Data-parallel over batch (4096 rows/core). The 4 gate projections run at
mixed precision chosen so the total error stays ~1.5e-2 (< 2e-2 gate):
  - i, f gates: full-K fp8e4m3 DoubleRow matmuls (2x PE rate)
  - o gate:     fp8 DoubleRow on K 0:256, bf16 on K 256:1024
  - g (tanh) gate: full bf16 (its error coefficient is 3.6x the others)
All W variants are premultiplied by 16 on the host (fp8 subnormal
avoidance); the activation instructions fold the 1/16 dequant into their
scale operand. PSUM chains mix fp8/bf16 matmuls at a consistent scale.

PE work: 9/16 of the GEMM at 2x rate -> ~157us/core vs 218us all-bf16.

Self-contained: shapes/sharding hardcoded for
input [32768, 1024], cell_state [32768, 512], W* [1024, 512].
"""

import os

import numpy as np
import ml_dtypes

import bass_rust
import concourse.bass as bass
import concourse.mybir as mybir
import concourse.tile as tile
from concourse.bass_utils import run_bass_kernel_spmd

N_CORES = 8
B = 32768
D = 1024
H = 512
P = 128
B_LOC = B // N_CORES        # 4096 rows per core
KO = D // P                 # 8 k-subtiles of 128
KS = KO // 2                # 4 DoubleRow k-steps of 256
NW = 4 * H                  # 2048 concatenated gate dim
NG = 4
BG_ROWS = 512               # batch rows per x slab
BG = B_LOC // BG_ROWS       # 8 slabs per core
BT_PER_BG = BG_ROWS // P    # 4 batch tiles per slab

WSCALE = 16.0               # host premultiplies all W; undone in activation

F8 = mybir.dt.float8e4
BF16 = mybir.dt.bfloat16
F32 = mybir.dt.float32
NPF8 = ml_dtypes.float8_e4m3
NPBF = ml_dtypes.bfloat16

LAST_RESULTS = None
_CACHED = {}


def _split_multi_waits(nc):
    """Legalize for a walrus build that accepts one sync-wait per instruction."""
    n = 0
    for f in nc.m.functions:
        for blk in f.blocks:
            insts = blk.instructions
            if not any(
                i.sync_info is not None and len(i.sync_info.on_wait) > 1
                for i in insts
            ):
                continue
            out = []
            for inst in insts:
                si = inst.sync_info
                if si is not None and len(si.on_wait) > 1:
                    waits = list(si.on_wait)
                    for w in waits[:-1]:
                        nop = mybir.InstNoOp(name=f"waitsplit_{n}", ins=[], outs=[])
                        n += 1
                        nop.engine = inst.engine
                        nop.sync_info = bass_rust.SyncInfo(on_wait=[w], on_update=[])
                        out.append(nop)
                    inst.sync_info = bass_rust.SyncInfo(
                        on_wait=[waits[-1]], on_update=list(si.on_update)
                    )
                out.append(inst)
            blk.instructions = out


class _FastTailTileContext(tile.TileContext):
    """Drop both tail all-engine barriers.

    The stock tail is [drain+waits][barrier][sem/queue reset][barrier].  The
    drain's sem waits already cover completion of every instruction and DMA,
    so by the time the gpsimd-side reset runs nothing is in flight that could
    observe the cleared semaphores; NRT waits for each engine stream to halt
    independently.  Saves ~8-10us of EVSEM barrier ring.
    """

    def _drain_and_barrier(self, tick_clock, wait_clock):
        # No global-clock sem waits on the drain: every out-DMA trigger on
        # the sync queue already waits for its producer and precedes the
        # drain, so all engine work is transitively complete when the drain
        # runs; the drain itself waits for the DMA rings to empty.
        drain_inst = self.nc.sync.drain()
        # Chain the gpsimd-side reset directly behind the drain (gpsimd has
        # been idle since the warmup memset; without this it would clear
        # live semaphores immediately).
        tail_sem = self.nc.alloc_semaphore("fast_tail_sem")
        drain_inst.then_inc(tail_sem)
        self.nc.gpsimd.wait_ge(tail_sem, 1)
        assert self.sems is not None
        popped = self.nc._tile_sem_poison_stack.pop()
        assert popped is self._sem_poison
        self.nc.clear_and_free_semaphores(list(self.sems.allocated().values()))


def _build(with_bias):
    nc = bass.Bass()
    AF = mybir.ActivationFunctionType
    ts = bass.ts
    DR = mybir.MatmulPerfMode.DoubleRow
    SCL = 1.0 / WSCALE

    x8t = nc.dram_tensor("x8t", [BG, P, KO, BG_ROWS], F8, kind="ExternalInput")
    xbt = nc.dram_tensor("xbt", [BG, P, KO, BG_ROWS], BF16, kind="ExternalInput")
    w8 = nc.dram_tensor("w8", [P, KO, 2 * H], F8, kind="ExternalInput")   # i|f
    w8o = nc.dram_tensor("w8o", [P, 4, H], F8, kind="ExternalInput")      # o k<512
    wb = nc.dram_tensor("wb", [P, KO, 2 * H], BF16, kind="ExternalInput")  # g|o
    cell = nc.dram_tensor("cell", [B_LOC, H], BF16, kind="ExternalInput")
    if with_bias:
        bias = nc.dram_tensor("bias", [P, NW], F32, kind="ExternalInput")
    h_out = nc.dram_tensor("h_out", [B_LOC, H], BF16, kind="ExternalOutput")
    c_out = nc.dram_tensor("c_out", [B_LOC, H], BF16, kind="ExternalOutput")

    with _FastTailTileContext(nc) as tc:
        with (
            tc.tile_pool(name="wpool", bufs=1) as wpool,
            tc.tile_pool(name="x8pool", bufs=3) as x8pool,
            tc.tile_pool(name="xbpool", bufs=3) as xbpool,
            tc.tile_pool(name="cpool", bufs=4) as cpool,
            tc.tile_pool(name="gpool", bufs=3) as gpool,
            tc.tile_pool(name="ppool", bufs=8, space="PSUM") as ppool,
        ):
            # PE warmup first: runs while the startup DMA triggers issue, so
            # the HAM clock gate opens before real data lands.
            wz = wpool.tile([P, P], F8, tag="wz", name="wz")
            nc.gpsimd.memset(wz[:], 0.0)
            warm_ps = ppool.tile([P, P], F32, tag="ps", name="warm_ps")
            for _ in range(16):
                nc.tensor.matmul(warm_ps[:], wz[:], wz[:], start=True, stop=True)

            bias_t = None
            if with_bias:
                bias_t = wpool.tile([P, NW], F32, tag="bias_t", name="bias_t")
                nc.sync.dma_start(bias_t[:], bias[:])

            # Resident weight tiles + slab-0 x, DMA'd in consumption order:
            # fp8 stream first (slab 0 runs its fp8 phase first), then the
            # bf16 stream, with the first pair's cell tiles slotted where the
            # epilogues need them.
            w8t = wpool.tile([P, KO, 2 * H], F8, tag="w8t", name="w8t")
            w8ot = wpool.tile([P, 4, H], F8, tag="w8ot", name="w8ot")
            wbt = wpool.tile([P, KO, 2 * H], BF16, tag="wbt", name="wbt")
            x8s0 = wpool.tile([P, KO, BG_ROWS], F8, tag="x8s0", name="x8s0")
            xbs0 = wpool.tile([P, KO, BG_ROWS], BF16, tag="xbs0", name="xbs0")
            cts0 = [
                cpool.tile([P, H], BF16, tag="ct", name=f"ct0_{j}")
                for j in range(BT_PER_BG)
            ]
            for k in range(KO):
                nc.sync.dma_start(x8s0[:, k, :], x8t[0, :, k, :])
                nc.sync.dma_start(w8t[:, k, :], w8[:, k, :])
                if k == 1:
                    # o's fp8 weights ride interleaved with kstep 0/1's i/f
                    # chunks — o's DR matmuls consume them at ~1.5us.
                    nc.sync.dma_start(w8ot[:, 0:2, :], w8o[:, 0:2, :])
                    # Priority gap: the chunks above gate the fp8 phase.
                    for dd in range(2):
                        scr = wpool.tile([P, 16], F8, tag=f"scr{dd}", name=f"scr{dd}")
                        nc.sync.dma_start(scr[:], x8t[0, :, 0, :16])
                if k == 3:
                    nc.sync.dma_start(w8ot[:, 2:4, :], w8o[:, 2:4, :])
            for k in range(KO):
                nc.sync.dma_start(xbs0[:, k, :], xbt[0, :, k, :])
                nc.sync.dma_start(wbt[:, k, :H], wb[:, k, :H])  # g columns
            for j in (0, 1):
                nc.sync.dma_start(cts0[j][:], cell[j * P : (j + 1) * P, :])
            for k in range(4, KO):
                nc.sync.dma_start(wbt[:, k, H:], wb[:, k, H:])  # o columns
            for j in (2, 3):
                nc.sync.dma_start(cts0[j][:], cell[j * P : (j + 1) * P, :])

            def mm_btile(ps, x8s, xbs, j):
                """i,f full fp8; o fp8 K<512 + bf16 k4..7; g full bf16."""
                for ks in range(KS):
                    l8 = x8s[:, 2 * ks : 2 * ks + 2, ts(j, P)]
                    for gi in (0, 1):  # i then f
                        for c in range(2):
                            nc.tensor.matmul(
                                ps[gi][:, ts(c, 256)],
                                l8,
                                w8t[:, 2 * ks : 2 * ks + 2,
                                    gi * H + c * 256 : gi * H + (c + 1) * 256],
                                start=(ks == 0 and c == 0),
                                stop=(ks == KS - 1),
                                perf_mode=DR,
                                skip_group_check=True,
                            )
                    if ks < 2:
                        for c in range(2):  # o's fp8 steps ride i/f's lhsT
                            nc.tensor.matmul(
                                ps[3][:, ts(c, 256)],
                                l8,
                                w8ot[:, 2 * ks : 2 * ks + 2,
                                     c * 256 : (c + 1) * 256],
                                start=(ks == 0 and c == 0),
                                stop=False,
                                perf_mode=DR,
                                skip_group_check=True,
                            )
                for k in range(KO):
                    lb = xbs[:, k, ts(j, P)]
                    nc.tensor.matmul(
                        ps[2], lb, wbt[:, k, :H], start=(k == 0), stop=(k == KO - 1)
                    )
                    if k >= 4:
                        nc.tensor.matmul(
                            ps[3], lb, wbt[:, k, H:],
                            start=False, stop=(k == KO - 1),
                            skip_group_check=True,
                        )

            def epilogue(ps, ct, rows, uid, splits=1):
                # psum order: 0=i 1=f 2=g 3=o. activation scale undoes the
                # host-side 16x W premultiply.
                if with_bias:
                    zs = []
                    for nn in range(NG):
                        z = gpool.tile([P, H], F32, tag=f"z{nn}", name=f"z{nn}_{uid}")
                        nc.vector.tensor_add(z[:], ps[nn], bias_t[:, ts(nn, H)])
                        zs.append(z)
                else:
                    zs = ps
                w_ = H // splits
                for q in range(splits):
                    cs = slice(q * w_, (q + 1) * w_)
                    # all four activations first: a gate's PSUM bank frees at
                    # its activation read, so front-loading them unblocks the
                    # bank rotation for btile N+2 ~2us earlier.
                    i_t = gpool.tile([P, w_], F32, tag="i_t", name=f"i_{uid}_{q}")
                    nc.scalar.activation(i_t[:], zs[0][:, cs], AF.Sigmoid, scale=SCL)
                    f_t = gpool.tile([P, w_], F32, tag="f_t", name=f"f_{uid}_{q}")
                    nc.scalar.activation(f_t[:], zs[1][:, cs], AF.Sigmoid, scale=SCL)
                    g_t = gpool.tile([P, w_], F32, tag="g_t", name=f"g_{uid}_{q}")
                    nc.scalar.activation(g_t[:], zs[2][:, cs], AF.Tanh, scale=SCL)
                    o_t = gpool.tile([P, w_], F32, tag="o_t", name=f"o_{uid}_{q}")
                    nc.scalar.activation(o_t[:], zs[3][:, cs], AF.Sigmoid, scale=SCL)

                    fc = gpool.tile([P, w_], F32, tag="fc", name=f"fc_{uid}_{q}")
                    nc.vector.tensor_mul(fc[:], f_t[:], ct[:, cs])
                    ig = gpool.tile([P, w_], F32, tag="ig", name=f"ig_{uid}_{q}")
                    nc.vector.tensor_mul(ig[:], i_t[:], g_t[:])
                    cn = gpool.tile([P, w_], BF16, tag="cn", name=f"cn_{uid}_{q}")
                    nc.vector.tensor_add(cn[:], fc[:], ig[:])
                    tn = gpool.tile([P, w_], F32, tag="tn", name=f"tn_{uid}_{q}")
                    nc.scalar.activation(tn[:], cn[:], AF.Tanh)
                    hn = gpool.tile([P, w_], BF16, tag="hn", name=f"hn_{uid}_{q}")
                    nc.vector.tensor_mul(hn[:], o_t[:], tn[:])

                    nc.sync.dma_start(c_out[rows, cs], cn[:])
                    nc.sync.dma_start(h_out[rows, cs], hn[:])
                return cn

            # Slab 0: j-pairs; within a pair run the fp8 phase first, then g,
            # then o's bf16 tail — matching the DMA stream arrival order.
            xslabs = {}

            def prefetch_slab(g):
                # The dma triggers sit on the sequential sync queue behind the
                # preceding epilogue's out-DMA triggers, which wait on that
                # epilogue's results — a natural throttle that keeps prefetch
                # from starving the slab-0 startup stream.
                x8s = x8pool.tile([P, KO, BG_ROWS], F8, tag="x8s", name=f"x8s_{g}")
                nc.sync.dma_start(x8s[:], x8t[g])
                xbs = xbpool.tile([P, KO, BG_ROWS], BF16, tag="xbs", name=f"xbs_{g}")
                nc.sync.dma_start(xbs[:], xbt[g])
                xslabs[g] = (x8s, xbs)

            for jp in (0, 2):
                ps2 = {
                    (j, nn): ppool.tile([P, H], F32, tag="ps", name=f"ps0_{j}_{nn}")
                    for j in (jp, jp + 1)
                    for nn in range(NG)
                }
                # phase 1: fp8 i,f (+ o's fp8 steps, carrying o's chain start)
                for ks in range(KS):
                    for j in (jp, jp + 1):
                        l8 = x8s0[:, 2 * ks : 2 * ks + 2, ts(j, P)]
                        for gi in (0, 1):
                            for c in range(2):
                                nc.tensor.matmul(
                                    ps2[(j, gi)][:, ts(c, 256)],
                                    l8,
                                    w8t[:, 2 * ks : 2 * ks + 2,
                                        gi * H + c * 256 : gi * H + (c + 1) * 256],
                                    start=(ks == 0 and c == 0),
                                    stop=(ks == KS - 1),
                                    perf_mode=DR,
                                    skip_group_check=True,
                                )
                        if ks < 2:
                            for c in range(2):
                                nc.tensor.matmul(
                                    ps2[(j, 3)][:, ts(c, 256)],
                                    l8,
                                    w8ot[:, 2 * ks : 2 * ks + 2,
                                         c * 256 : (c + 1) * 256],
                                    start=(ks == 0 and c == 0),
                                    stop=False,
                                    perf_mode=DR,
                                    skip_group_check=True,
                                )
                # phase 2: gate g, k-major across the pair
                for k in range(KO):
                    for j in (jp, jp + 1):
                        nc.tensor.matmul(
                            ps2[(j, 2)], xbs0[:, k, ts(j, P)], wbt[:, k, :H],
                            start=(k == 0), stop=(k == KO - 1),
                        )
                # phase 3: gate o bf16 tail
                for k in range(4, KO):
                    for j in (jp, jp + 1):
                        nc.tensor.matmul(
                            ps2[(j, 3)], xbs0[:, k, ts(j, P)], wbt[:, k, H:],
                            start=False, stop=(k == KO - 1),
                            skip_group_check=True,
                        )
                for j in (jp, jp + 1):
                    epilogue(
                        [ps2[(j, nn)] for nn in range(NG)],
                        cts0[j],
                        slice(j * P, (j + 1) * P),
                        f"g0_{j}",
                    )
                    if j == 1:
                        prefetch_slab(1)
                    elif j == 3:
                        prefetch_slab(2)

            # Slabs 1..7 against prefetched slabs (next slab requested after
            # this slab's first epilogue — ~3 btiles of lead time).  Cell
            # tiles load one btile ahead: issuing the DMA before the current
            # epilogue keeps its trigger clear of the out-DMA triggers
            # (which block the sync queue on epilogue results).
            ct_next = cpool.tile([P, H], BF16, tag="ct", name="ct_4")
            nc.sync.dma_start(ct_next[:], cell[4 * P : 5 * P, :])
            for g in range(1, BG):
                x8s, xbs = xslabs[g]
                for j in range(BT_PER_BG):
                    bt = g * BT_PER_BG + j
                    rows = slice(bt * P, (bt + 1) * P)
                    ct = ct_next
                    ps = [
                        ppool.tile([P, H], F32, tag="ps", name=f"ps{nn}_{bt}")
                        for nn in range(NG)
                    ]
                    mm_btile(ps, x8s, xbs, j)
                    if bt + 1 < BG * BT_PER_BG:
                        ct_next = cpool.tile(
                            [P, H], BF16, tag="ct", name=f"ct_{bt + 1}"
                        )
                        nc.sync.dma_start(
                            ct_next[:], cell[(bt + 1) * P : (bt + 2) * P, :]
                        )
                    last_bt = g == BG - 1 and j == BT_PER_BG - 1
                    epilogue(ps, ct, rows, f"g{g}_{j}", splits=2 if last_bt else 1)
                    if j == 0 and g + 1 < BG:
                        prefetch_slab(g + 1)

    _split_multi_waits(nc)
    return nc


def kernel(input, cell_state, Wi, bi, Wf, bf, Wg, bg, Wo, bo):
    global LAST_RESULTS

    x = np.asarray(input, dtype=np.float32)
    cell = np.ascontiguousarray(np.asarray(cell_state, dtype=np.float32).astype(NPBF))
    Wi, Wf, Wg, Wo = (np.asarray(m, dtype=np.float32) for m in (Wi, Wf, Wg, Wo))
    bcat = np.concatenate(
        [np.asarray(v, dtype=np.float32) for v in (bi, bf, bg, bo)]
    )  # [4H] in i,f,g,o order (matches psum order)
    with_bias = bool(np.any(bcat))

    def wlay(Wcat, np_dt):
        # [D, N] -> [p, ko, n], scaled by WSCALE
        n = Wcat.shape[1]
        return np.ascontiguousarray(
            (Wcat * WSCALE).astype(np_dt).reshape(KO, P, n).transpose(1, 0, 2)
        )

    w8_dev = wlay(np.concatenate([Wi, Wf], axis=1), NPF8)          # [P,KO,1024]
    wb_dev = wlay(np.concatenate([Wg, Wo], axis=1), NPBF)          # [P,KO,1024]
    w8o_dev = np.ascontiguousarray(
        (Wo[:512] * WSCALE).astype(NPF8).reshape(4, P, H).transpose(1, 0, 2)
    )  # [P,4,H]

    in_maps = []
    for c in range(N_CORES):
        xc = x[c * B_LOC : (c + 1) * B_LOC]  # [4096, 1024]
        def xlay(np_dt):
            return np.ascontiguousarray(
                xc.astype(np_dt)
                .reshape(BG, BG_ROWS, KO, P)
                .transpose(0, 3, 2, 1)
            )
        m = {
            "x8t": xlay(NPF8),
            "xbt": xlay(NPBF),
            "w8": w8_dev,
            "w8o": w8o_dev,
            "wb": wb_dev,
            "cell": cell[c * B_LOC : (c + 1) * B_LOC],
        }
        if with_bias:
            m["bias"] = np.ascontiguousarray(
                np.broadcast_to(bcat[None, :] * WSCALE, (P, NW)).astype(np.float32)
            )
        in_maps.append(m)

    key = with_bias
    if key not in _CACHED:
        _CACHED[key] = _build(with_bias)
    nc = _CACHED[key]

    trace = os.environ.get("KERNEL_TRACE", "0") == "1"
    res = run_bass_kernel_spmd(nc, in_maps, list(range(N_CORES)), trace=trace)
    LAST_RESULTS = res

    h = np.concatenate(
        [res.results[c]["h_out"].astype(np.float32) for c in range(N_CORES)], axis=0
    )
    c_ = np.concatenate(
        [res.results[c]["c_out"].astype(np.float32) for c in range(N_CORES)], axis=0
    )
    return h, c_


# revision 6
# speedup vs baseline: 1.0599x; 1.0037x over previous
"""CustomLSTM cell on 8 TRN2 NeuronCores — fp8/bf16 hybrid matmul.

Data-parallel over batch (4096 rows/core). The 4 gate projections run at
mixed precision chosen so the total error stays ~1.5e-2 (< 2e-2 gate):
  - i, f gates: full-K fp8e4m3 DoubleRow matmuls (2x PE rate)
  - o gate:     fp8 DoubleRow on K 0:256, bf16 on K 256:1024
  - g (tanh) gate: full bf16 (its error coefficient is 3.6x the others)
All W variants are premultiplied by 16 on the host (fp8 subnormal
avoidance); the activation instructions fold the 1/16 dequant into their
scale operand. PSUM chains mix fp8/bf16 matmuls at a consistent scale.

PE work: 9/16 of the GEMM at 2x rate -> ~157us/core vs 218us all-bf16.

Self-contained: shapes/sharding hardcoded for
input [32768, 1024], cell_state [32768, 512], W* [1024, 512].
"""

import os

import numpy as np
import ml_dtypes

import bass_rust
import concourse.bass as bass
import concourse.mybir as mybir
import concourse.tile as tile
from concourse.bass_utils import run_bass_kernel_spmd

N_CORES = 8
B = 32768
D = 1024
H = 512
P = 128
B_LOC = B // N_CORES        # 4096 rows per core
KO = D // P                 # 8 k-subtiles of 128
KS = KO // 2                # 4 DoubleRow k-steps of 256
NW = 4 * H                  # 2048 concatenated gate dim
NG = 4
BG_ROWS = 512               # batch rows per x slab
BG = B_LOC // BG_ROWS       # 8 slabs per core
BT_PER_BG = BG_ROWS // P    # 4 batch tiles per slab

WSCALE = 16.0               # host premultiplies all W; undone in activation

F8 = mybir.dt.float8e4
BF16 = mybir.dt.bfloat16
F32 = mybir.dt.float32
NPF8 = ml_dtypes.float8_e4m3
NPBF = ml_dtypes.bfloat16

LAST_RESULTS = None
_CACHED = {}


def _split_multi_waits(nc):
    """Legalize for a walrus build that accepts one sync-wait per instruction."""
    n = 0
    for f in nc.m.functions:
        for blk in f.blocks:
            insts = blk.instructions
            if not any(
                i.sync_info is not None and len(i.sync_info.on_wait) > 1
                for i in insts
            ):
                continue
            out = []
            for inst in insts:
                si = inst.sync_info
                if si is not None and len(si.on_wait) > 1:
                    waits = list(si.on_wait)
                    for w in waits[:-1]:
                        nop = mybir.InstNoOp(name=f"waitsplit_{n}", ins=[], outs=[])
                        n += 1
                        nop.engine = inst.engine
                        nop.sync_info = bass_rust.SyncInfo(on_wait=[w], on_update=[])
                        out.append(nop)
                    inst.sync_info = bass_rust.SyncInfo(
                        on_wait=[waits[-1]], on_update=list(si.on_update)
                    )
                out.append(inst)
            blk.instructions = out


class _FastTailTileContext(tile.TileContext):
    """Drop both tail all-engine barriers.

    The stock tail is [drain+waits][barrier][sem/queue reset][barrier].  The
    drain's sem waits already cover completion of every instruction and DMA,
    so by the time the gpsimd-side reset runs nothing is in flight that could
    observe the cleared semaphores; NRT waits for each engine stream to halt
    independently.  Saves ~8-10us of EVSEM barrier ring.
    """

    def _drain_and_barrier(self, tick_clock, wait_clock):
        # No global-clock sem waits on the drain: every out-DMA trigger on
        # the sync queue already waits for its producer and precedes the
        # drain, so all engine work is transitively complete when the drain
        # runs; the drain itself waits for the DMA rings to empty.
        drain_inst = self.nc.sync.drain()
        # Chain the gpsimd-side reset directly behind the drain (gpsimd has
        # been idle since the warmup memset; without this it would clear
        # live semaphores immediately).
        tail_sem = self.nc.alloc_semaphore("fast_tail_sem")
        drain_inst.then_inc(tail_sem)
        self.nc.gpsimd.wait_ge(tail_sem, 1)
        assert self.sems is not None
        popped = self.nc._tile_sem_poison_stack.pop()
        assert popped is self._sem_poison
        self.nc.clear_and_free_semaphores(list(self.sems.allocated().values()))


def _build(with_bias):
    nc = bass.Bass()
    AF = mybir.ActivationFunctionType
    ts = bass.ts
    DR = mybir.MatmulPerfMode.DoubleRow
    SCL = 1.0 / WSCALE

    x8t = nc.dram_tensor("x8t", [BG, P, KO, BG_ROWS], F8, kind="ExternalInput")
    xbt = nc.dram_tensor("xbt", [BG, P, KO, BG_ROWS], BF16, kind="ExternalInput")
    w8 = nc.dram_tensor("w8", [P, KO, 2 * H], F8, kind="ExternalInput")   # i|f
    w8o = nc.dram_tensor("w8o", [P, 4, H], F8, kind="ExternalInput")      # o k<512
    wb = nc.dram_tensor("wb", [2, P, KO, H], BF16, kind="ExternalInput")  # [g,o]
    cell = nc.dram_tensor("cell", [B_LOC, H], BF16, kind="ExternalInput")
    if with_bias:
        bias = nc.dram_tensor("bias", [P, NW], F32, kind="ExternalInput")
    h_out = nc.dram_tensor("h_out", [B_LOC, H], BF16, kind="ExternalOutput")
    c_out = nc.dram_tensor("c_out", [B_LOC, H], BF16, kind="ExternalOutput")

    with _FastTailTileContext(nc) as tc:
        with (
            tc.tile_pool(name="wpool", bufs=1) as wpool,
            tc.tile_pool(name="x8pool", bufs=3) as x8pool,
            tc.tile_pool(name="xbpool", bufs=3) as xbpool,
            tc.tile_pool(name="cpool", bufs=4) as cpool,
            tc.tile_pool(name="gpool", bufs=3) as gpool,
            tc.tile_pool(name="ppool", bufs=8, space="PSUM") as ppool,
        ):
            # PE warmup first: runs while the startup DMA triggers issue, so
            # the HAM clock gate opens before real data lands.
            wz = wpool.tile([P, P], F8, tag="wz", name="wz")
            nc.gpsimd.memset(wz[:], 0.0)
            warm_ps = ppool.tile([P, P], F32, tag="ps", name="warm_ps")
            for _ in range(16):
                nc.tensor.matmul(warm_ps[:], wz[:], wz[:], start=True, stop=True)

            bias_t = None
            if with_bias:
                bias_t = wpool.tile([P, NW], F32, tag="bias_t", name="bias_t")
                nc.sync.dma_start(bias_t[:], bias[:])

            # Resident weight tiles + slab-0 x, DMA'd in consumption order:
            # fp8 stream first (slab 0 runs its fp8 phase first), then the
            # bf16 stream, with the first pair's cell tiles slotted where the
            # epilogues need them.
            w8t = wpool.tile([P, KO, 2 * H], F8, tag="w8t", name="w8t")
            w8ot = wpool.tile([P, 4, H], F8, tag="w8ot", name="w8ot")
            wgt = wpool.tile([P, KO, H], BF16, tag="wgt", name="wgt")
            wot = wpool.tile([P, KO, H], BF16, tag="wot", name="wot")
            x8s0 = wpool.tile([P, KO, BG_ROWS], F8, tag="x8s0", name="x8s0")
            xbs0 = wpool.tile([P, KO, BG_ROWS], BF16, tag="xbs0", name="xbs0")
            cts0 = [
                cpool.tile([P, H], BF16, tag="ct", name=f"ct0_{j}")
                for j in range(BT_PER_BG)
            ]
            # Half-slab granularity: the dram layouts are k-contiguous per
            # partition, so each half is 128 large descriptors instead of
            # 4x128 small ones — the sync queue's descriptor generation was
            # pacing the startup stream.
            for h2 in range(2):
                ks_ = slice(4 * h2, 4 * h2 + 4)
                nc.sync.dma_start(x8s0[:, ks_, :], x8t[0, :, ks_, :])
                nc.sync.dma_start(w8t[:, ks_, :], w8[:, ks_, :])
                nc.sync.dma_start(w8ot[:, 2 * h2 : 2 * h2 + 2, :],
                                  w8o[:, 2 * h2 : 2 * h2 + 2, :])
            for h2 in range(2):
                ks_ = slice(4 * h2, 4 * h2 + 4)
                nc.sync.dma_start(xbs0[:, ks_, :], xbt[0, :, ks_, :])
                nc.sync.dma_start(wgt[:, ks_, :], wb[0, :, ks_, :])
            for j in (0, 1):
                nc.sync.dma_start(cts0[j][:], cell[j * P : (j + 1) * P, :])
            nc.sync.dma_start(wot[:, 4:, :], wb[1, :, 4:, :])  # o k4..7
            for j in (2, 3):
                nc.sync.dma_start(cts0[j][:], cell[j * P : (j + 1) * P, :])

            def mm_btile(ps, x8s, xbs, j):
                """i,f full fp8; o fp8 K<512 + bf16 k4..7; g full bf16."""
                for ks in range(KS):
                    l8 = x8s[:, 2 * ks : 2 * ks + 2, ts(j, P)]
                    for gi in (0, 1):  # i then f
                        for c in range(2):
                            nc.tensor.matmul(
                                ps[gi][:, ts(c, 256)],
                                l8,
                                w8t[:, 2 * ks : 2 * ks + 2,
                                    gi * H + c * 256 : gi * H + (c + 1) * 256],
                                start=(ks == 0 and c == 0),
                                stop=(ks == KS - 1),
                                perf_mode=DR,
                                skip_group_check=True,
                            )
                    if ks < 2:
                        for c in range(2):  # o's fp8 steps ride i/f's lhsT
                            nc.tensor.matmul(
                                ps[3][:, ts(c, 256)],
                                l8,
                                w8ot[:, 2 * ks : 2 * ks + 2,
                                     c * 256 : (c + 1) * 256],
                                start=(ks == 0 and c == 0),
                                stop=False,
                                perf_mode=DR,
                                skip_group_check=True,
                            )
                for k in range(KO):
                    lb = xbs[:, k, ts(j, P)]
                    nc.tensor.matmul(
                        ps[2], lb, wgt[:, k, :], start=(k == 0), stop=(k == KO - 1)
                    )
                    if k >= 4:
                        nc.tensor.matmul(
                            ps[3], lb, wot[:, k, :],
                            start=False, stop=(k == KO - 1),
                            skip_group_check=True,
                        )

            def epilogue(ps, ct, rows, uid, splits=1):
                # psum order: 0=i 1=f 2=g 3=o. activation scale undoes the
                # host-side 16x W premultiply.
                if with_bias:
                    zs = []
                    for nn in range(NG):
                        z = gpool.tile([P, H], F32, tag=f"z{nn}", name=f"z{nn}_{uid}")
                        nc.vector.tensor_add(z[:], ps[nn], bias_t[:, ts(nn, H)])
                        zs.append(z)
                else:
                    zs = ps
                w_ = H // splits
                for q in range(splits):
                    cs = slice(q * w_, (q + 1) * w_)
                    # all four activations first: a gate's PSUM bank frees at
                    # its activation read, so front-loading them unblocks the
                    # bank rotation for btile N+2 ~2us earlier.
                    i_t = gpool.tile([P, w_], F32, tag="i_t", name=f"i_{uid}_{q}")
                    nc.scalar.activation(i_t[:], zs[0][:, cs], AF.Sigmoid, scale=SCL)
                    f_t = gpool.tile([P, w_], F32, tag="f_t", name=f"f_{uid}_{q}")
                    nc.scalar.activation(f_t[:], zs[1][:, cs], AF.Sigmoid, scale=SCL)
                    g_t = gpool.tile([P, w_], F32, tag="g_t", name=f"g_{uid}_{q}")
                    nc.scalar.activation(g_t[:], zs[2][:, cs], AF.Tanh, scale=SCL)
                    o_t = gpool.tile([P, w_], F32, tag="o_t", name=f"o_{uid}_{q}")
                    nc.scalar.activation(o_t[:], zs[3][:, cs], AF.Sigmoid, scale=SCL)

                    fc = gpool.tile([P, w_], F32, tag="fc", name=f"fc_{uid}_{q}")
                    nc.vector.tensor_mul(fc[:], f_t[:], ct[:, cs])
                    ig = gpool.tile([P, w_], F32, tag="ig", name=f"ig_{uid}_{q}")
                    nc.vector.tensor_mul(ig[:], i_t[:], g_t[:])
                    cn = gpool.tile([P, w_], BF16, tag="cn", name=f"cn_{uid}_{q}")
                    nc.vector.tensor_add(cn[:], fc[:], ig[:])
                    tn = gpool.tile([P, w_], F32, tag="tn", name=f"tn_{uid}_{q}")
                    nc.scalar.activation(tn[:], cn[:], AF.Tanh)
                    hn = gpool.tile([P, w_], BF16, tag="hn", name=f"hn_{uid}_{q}")
                    nc.vector.tensor_mul(hn[:], o_t[:], tn[:])

                    nc.sync.dma_start(c_out[rows, cs], cn[:])
                    nc.sync.dma_start(h_out[rows, cs], hn[:])
                return cn

            # Slab 0: j-pairs; within a pair run the fp8 phase first, then g,
            # then o's bf16 tail — matching the DMA stream arrival order.
            xslabs = {}

            def prefetch_slab(g):
                # The dma triggers sit on the sequential sync queue behind the
                # preceding epilogue's out-DMA triggers, which wait on that
                # epilogue's results — a natural throttle that keeps prefetch
                # from starving the slab-0 startup stream.
                x8s = x8pool.tile([P, KO, BG_ROWS], F8, tag="x8s", name=f"x8s_{g}")
                nc.sync.dma_start(x8s[:], x8t[g])
                xbs = xbpool.tile([P, KO, BG_ROWS], BF16, tag="xbs", name=f"xbs_{g}")
                nc.sync.dma_start(xbs[:], xbt[g])
                xslabs[g] = (x8s, xbs)

            for jp in (0, 2):
                ps2 = {
                    (j, nn): ppool.tile([P, H], F32, tag="ps", name=f"ps0_{j}_{nn}")
                    for j in (jp, jp + 1)
                    for nn in range(NG)
                }
                # phase 1: fp8 i,f (+ o's fp8 steps, carrying o's chain start)
                for ks in range(KS):
                    for j in (jp, jp + 1):
                        l8 = x8s0[:, 2 * ks : 2 * ks + 2, ts(j, P)]
                        for gi in (0, 1):
                            for c in range(2):
                                nc.tensor.matmul(
                                    ps2[(j, gi)][:, ts(c, 256)],
                                    l8,
                                    w8t[:, 2 * ks : 2 * ks + 2,
                                        gi * H + c * 256 : gi * H + (c + 1) * 256],
                                    start=(ks == 0 and c == 0),
                                    stop=(ks == KS - 1),
                                    perf_mode=DR,
                                    skip_group_check=True,
                                )
                        if ks < 2:
                            for c in range(2):
                                nc.tensor.matmul(
                                    ps2[(j, 3)][:, ts(c, 256)],
                                    l8,
                                    w8ot[:, 2 * ks : 2 * ks + 2,
                                         c * 256 : (c + 1) * 256],
                                    start=(ks == 0 and c == 0),
                                    stop=False,
                                    perf_mode=DR,
                                    skip_group_check=True,
                                )
                # phase 2: gate g, k-major across the pair
                for k in range(KO):
                    for j in (jp, jp + 1):
                        nc.tensor.matmul(
                            ps2[(j, 2)], xbs0[:, k, ts(j, P)], wgt[:, k, :],
                            start=(k == 0), stop=(k == KO - 1),
                        )
                # phase 3: gate o bf16 tail
                for k in range(4, KO):
                    for j in (jp, jp + 1):
                        nc.tensor.matmul(
                            ps2[(j, 3)], xbs0[:, k, ts(j, P)], wot[:, k, :],
                            start=False, stop=(k == KO - 1),
                            skip_group_check=True,
                        )
                for j in (jp, jp + 1):
                    epilogue(
                        [ps2[(j, nn)] for nn in range(NG)],
                        cts0[j],
                        slice(j * P, (j + 1) * P),
                        f"g0_{j}",
                    )
                    if j == 1:
                        prefetch_slab(1)
                    elif j == 3:
                        prefetch_slab(2)

            # Slabs 1..7 against prefetched slabs (next slab requested after
            # this slab's first epilogue — ~3 btiles of lead time).  Cell
            # tiles load one btile ahead: issuing the DMA before the current
            # epilogue keeps its trigger clear of the out-DMA triggers
            # (which block the sync queue on epilogue results).
            ct_next = cpool.tile([P, H], BF16, tag="ct", name="ct_4")
            nc.sync.dma_start(ct_next[:], cell[4 * P : 5 * P, :])
            for g in range(1, BG):
                x8s, xbs = xslabs[g]
                for j in range(BT_PER_BG):
                    bt = g * BT_PER_BG + j
                    rows = slice(bt * P, (bt + 1) * P)
                    ct = ct_next
                    ps = [
                        ppool.tile([P, H], F32, tag="ps", name=f"ps{nn}_{bt}")
                        for nn in range(NG)
                    ]
                    mm_btile(ps, x8s, xbs, j)
                    if bt + 1 < BG * BT_PER_BG:
                        ct_next = cpool.tile(
                            [P, H], BF16, tag="ct", name=f"ct_{bt + 1}"
                        )
                        nc.sync.dma_start(
                            ct_next[:], cell[(bt + 1) * P : (bt + 2) * P, :]
                        )
                    last_bt = g == BG - 1 and j == BT_PER_BG - 1
                    epilogue(ps, ct, rows, f"g{g}_{j}", splits=2 if last_bt else 1)
                    if j == 0 and g + 1 < BG:
                        prefetch_slab(g + 1)

    _split_multi_waits(nc)
    return nc


def kernel(input, cell_state, Wi, bi, Wf, bf, Wg, bg, Wo, bo):
    global LAST_RESULTS

    x = np.asarray(input, dtype=np.float32)
    cell = np.ascontiguousarray(np.asarray(cell_state, dtype=np.float32).astype(NPBF))
    Wi, Wf, Wg, Wo = (np.asarray(m, dtype=np.float32) for m in (Wi, Wf, Wg, Wo))
    bcat = np.concatenate(
        [np.asarray(v, dtype=np.float32) for v in (bi, bf, bg, bo)]
    )  # [4H] in i,f,g,o order (matches psum order)
    with_bias = bool(np.any(bcat))

    def wlay(Wcat, np_dt):
        # [D, N] -> [p, ko, n], scaled by WSCALE
        n = Wcat.shape[1]
        return np.ascontiguousarray(
            (Wcat * WSCALE).astype(np_dt).reshape(KO, P, n).transpose(1, 0, 2)
        )

    w8_dev = wlay(np.concatenate([Wi, Wf], axis=1), NPF8)          # [P,KO,1024]
    wb_dev = np.ascontiguousarray(
        np.stack([wlay(Wg, NPBF), wlay(Wo, NPBF)])
    )  # [2,P,KO,H]
    w8o_dev = np.ascontiguousarray(
        (Wo[:512] * WSCALE).astype(NPF8).reshape(4, P, H).transpose(1, 0, 2)
    )  # [P,4,H]

    in_maps = []
    for c in range(N_CORES):
        xc = x[c * B_LOC : (c + 1) * B_LOC]  # [4096, 1024]
        def xlay(np_dt):
            return np.ascontiguousarray(
                xc.astype(np_dt)
                .reshape(BG, BG_ROWS, KO, P)
                .transpose(0, 3, 2, 1)
            )
        m = {
            "x8t": xlay(NPF8),
            "xbt": xlay(NPBF),
            "w8": w8_dev,
            "w8o": w8o_dev,
            "wb": wb_dev,
            "cell": cell[c * B_LOC : (c + 1) * B_LOC],
        }
        if with_bias:
            m["bias"] = np.ascontiguousarray(
                np.broadcast_to(bcat[None, :] * WSCALE, (P, NW)).astype(np.float32)
            )
        in_maps.append(m)

    key = with_bias
    if key not in _CACHED:
        _CACHED[key] = _build(with_bias)
    nc = _CACHED[key]

    trace = os.environ.get("KERNEL_TRACE", "0") == "1"
    res = run_bass_kernel_spmd(nc, in_maps, list(range(N_CORES)), trace=trace)
    LAST_RESULTS = res

    h = np.concatenate(
        [res.results[c]["h_out"].astype(np.float32) for c in range(N_CORES)], axis=0
    )
    c_ = np.concatenate(
        [res.results[c]["c_out"].astype(np.float32) for c in range(N_CORES)], axis=0
    )
    return h, c_


# revision 7
# speedup vs baseline: 1.1001x; 1.0380x over previous
"""CustomLSTM cell on 8 TRN2 NeuronCores — fp8/bf16 hybrid matmul.

Data-parallel over batch (4096 rows/core). The 4 gate projections run at
mixed precision chosen so the total error stays ~1.5e-2 (< 2e-2 gate):
  - i, f gates: full-K fp8e4m3 DoubleRow matmuls (2x PE rate)
  - o gate:     fp8 DoubleRow on K 0:256, bf16 on K 256:1024
  - g (tanh) gate: full bf16 (its error coefficient is 3.6x the others)
All W variants are premultiplied by 16 on the host (fp8 subnormal
avoidance); the activation instructions fold the 1/16 dequant into their
scale operand. PSUM chains mix fp8/bf16 matmuls at a consistent scale.

PE work: 9/16 of the GEMM at 2x rate -> ~157us/core vs 218us all-bf16.

Self-contained: shapes/sharding hardcoded for
input [32768, 1024], cell_state [32768, 512], W* [1024, 512].
"""

import os

import numpy as np
import ml_dtypes

import bass_rust
import concourse.bass as bass
import concourse.mybir as mybir
import concourse.tile as tile
from concourse.bass_utils import run_bass_kernel_spmd

N_CORES = 8
B = 32768
D = 1024
H = 512
P = 128
B_LOC = B // N_CORES        # 4096 rows per core
KO = D // P                 # 8 k-subtiles of 128
KS = KO // 2                # 4 DoubleRow k-steps of 256
NW = 4 * H                  # 2048 concatenated gate dim
NG = 4
BG_ROWS = 512               # batch rows per x slab
BG = B_LOC // BG_ROWS       # 8 slabs per core
BT_PER_BG = BG_ROWS // P    # 4 batch tiles per slab

WSCALE = 16.0               # host premultiplies all W; undone in activation

F8 = mybir.dt.float8e4
BF16 = mybir.dt.bfloat16
F32 = mybir.dt.float32
NPF8 = ml_dtypes.float8_e4m3
NPBF = ml_dtypes.bfloat16

LAST_RESULTS = None
_CACHED = {}


def _split_multi_waits(nc):
    """Legalize for a walrus build that accepts one sync-wait per instruction."""
    n = 0
    for f in nc.m.functions:
        for blk in f.blocks:
            insts = blk.instructions
            if not any(
                i.sync_info is not None and len(i.sync_info.on_wait) > 1
                for i in insts
            ):
                continue
            out = []
            for inst in insts:
                si = inst.sync_info
                if si is not None and len(si.on_wait) > 1:
                    waits = list(si.on_wait)
                    for w in waits[:-1]:
                        nop = mybir.InstNoOp(name=f"waitsplit_{n}", ins=[], outs=[])
                        n += 1
                        nop.engine = inst.engine
                        nop.sync_info = bass_rust.SyncInfo(on_wait=[w], on_update=[])
                        out.append(nop)
                    inst.sync_info = bass_rust.SyncInfo(
                        on_wait=[waits[-1]], on_update=list(si.on_update)
                    )
                out.append(inst)
            blk.instructions = out


class _FastTailTileContext(tile.TileContext):
    """Drop both tail all-engine barriers.

    The stock tail is [drain+waits][barrier][sem/queue reset][barrier].  The
    drain's sem waits already cover completion of every instruction and DMA,
    so by the time the gpsimd-side reset runs nothing is in flight that could
    observe the cleared semaphores; NRT waits for each engine stream to halt
    independently.  Saves ~8-10us of EVSEM barrier ring.
    """

    def _drain_and_barrier(self, tick_clock, wait_clock):
        # No global-clock sem waits on the drain: every out-DMA trigger on
        # the sync queue already waits for its producer and precedes the
        # drain, so all engine work is transitively complete when the drain
        # runs; the drain itself waits for the DMA rings to empty.
        drain_inst = self.nc.sync.drain()
        # Chain the gpsimd-side reset directly behind the drain (gpsimd has
        # been idle since the warmup memset; without this it would clear
        # live semaphores immediately).
        tail_sem = self.nc.alloc_semaphore("fast_tail_sem")
        drain_inst.then_inc(tail_sem)
        self.nc.gpsimd.wait_ge(tail_sem, 1)
        assert self.sems is not None
        popped = self.nc._tile_sem_poison_stack.pop()
        assert popped is self._sem_poison
        self.nc.clear_and_free_semaphores(list(self.sems.allocated().values()))


def _build(with_bias):
    nc = bass.Bass()
    AF = mybir.ActivationFunctionType
    ts = bass.ts
    DR = mybir.MatmulPerfMode.DoubleRow
    SCL = 1.0 / WSCALE

    x8t = nc.dram_tensor("x8t", [BG, P, KO, BG_ROWS], F8, kind="ExternalInput")
    xbt = nc.dram_tensor("xbt", [BG, P, KO, BG_ROWS], BF16, kind="ExternalInput")
    w8 = nc.dram_tensor("w8", [P, KO, 2 * H], F8, kind="ExternalInput")   # i|f
    w8o = nc.dram_tensor("w8o", [P, 4, H], F8, kind="ExternalInput")      # o k<512
    wb = nc.dram_tensor("wb", [2, P, KO, H], BF16, kind="ExternalInput")  # [g,o]
    cell = nc.dram_tensor("cell", [B_LOC, H], BF16, kind="ExternalInput")
    if with_bias:
        bias = nc.dram_tensor("bias", [P, NW], F32, kind="ExternalInput")
    h_out = nc.dram_tensor("h_out", [B_LOC, H], BF16, kind="ExternalOutput")
    c_out = nc.dram_tensor("c_out", [B_LOC, H], BF16, kind="ExternalOutput")

    with _FastTailTileContext(nc) as tc:
        with (
            tc.tile_pool(name="wpool", bufs=1) as wpool,
            tc.tile_pool(name="x8pool", bufs=3) as x8pool,
            tc.tile_pool(name="xbpool", bufs=3) as xbpool,
            tc.tile_pool(name="cpool", bufs=4) as cpool,
            tc.tile_pool(name="gpool", bufs=3) as gpool,
            tc.tile_pool(name="ppool", bufs=8, space="PSUM") as ppool,
        ):
            # PE warmup first: runs while the startup DMA triggers issue, so
            # the HAM clock gate opens before real data lands.
            wz = wpool.tile([P, P], F8, tag="wz", name="wz")
            nc.gpsimd.memset(wz[:], 0.0)
            warm_ps = ppool.tile([P, P], F32, tag="ps", name="warm_ps")
            for _ in range(28):
                nc.tensor.matmul(warm_ps[:], wz[:], wz[:], start=True, stop=True)

            bias_t = None
            if with_bias:
                bias_t = wpool.tile([P, NW], F32, tag="bias_t", name="bias_t")
                nc.sync.dma_start(bias_t[:], bias[:])

            # Resident weight tiles + slab-0 x, DMA'd in consumption order:
            # fp8 stream first (slab 0 runs its fp8 phase first), then the
            # bf16 stream, with the first pair's cell tiles slotted where the
            # epilogues need them.
            w8t = wpool.tile([P, KO, 2 * H], F8, tag="w8t", name="w8t")
            w8ot = wpool.tile([P, 4, H], F8, tag="w8ot", name="w8ot")
            wgt = wpool.tile([P, KO, H], BF16, tag="wgt", name="wgt")
            wot = wpool.tile([P, KO, H], BF16, tag="wot", name="wot")
            x8s0 = wpool.tile([P, KO, BG_ROWS], F8, tag="x8s0", name="x8s0")
            xbs0 = wpool.tile([P, KO, BG_ROWS], BF16, tag="xbs0", name="xbs0")
            cts0 = [
                cpool.tile([P, H], BF16, tag="ct", name=f"ct0_{j}")
                for j in range(BT_PER_BG)
            ]
            # Half-slab granularity: the dram layouts are k-contiguous per
            # partition, so each half is 128 large descriptors instead of
            # 4x128 small ones — the sync queue's descriptor generation was
            # pacing the startup stream.
            for h2 in range(2):
                ks_ = slice(4 * h2, 4 * h2 + 4)
                nc.sync.dma_start(x8s0[:, ks_, :], x8t[0, :, ks_, :])
                nc.sync.dma_start(w8t[:, ks_, :], w8[:, ks_, :])
                nc.sync.dma_start(w8ot[:, 2 * h2 : 2 * h2 + 2, :],
                                  w8o[:, 2 * h2 : 2 * h2 + 2, :])
                nc.sync.dma_start(xbs0[:, ks_, :], xbt[0, :, ks_, :])
                nc.sync.dma_start(wgt[:, ks_, :], wb[0, :, ks_, :])
            for j in (0, 1):
                nc.sync.dma_start(cts0[j][:], cell[j * P : (j + 1) * P, :])
            nc.sync.dma_start(wot[:, 4:, :], wb[1, :, 4:, :])  # o k4..7
            for j in (2, 3):
                nc.sync.dma_start(cts0[j][:], cell[j * P : (j + 1) * P, :])

            def mm_btile(ps, x8s, xbs, j):
                """i,f full fp8; o fp8 K<512 + bf16 k4..7; g full bf16."""
                for ks in range(KS):
                    l8 = x8s[:, 2 * ks : 2 * ks + 2, ts(j, P)]
                    for gi in (0, 1):  # i then f
                        for c in range(2):
                            nc.tensor.matmul(
                                ps[gi][:, ts(c, 256)],
                                l8,
                                w8t[:, 2 * ks : 2 * ks + 2,
                                    gi * H + c * 256 : gi * H + (c + 1) * 256],
                                start=(ks == 0 and c == 0),
                                stop=(ks == KS - 1),
                                perf_mode=DR,
                                skip_group_check=True,
                            )
                    if ks < 2:
                        for c in range(2):  # o's fp8 steps ride i/f's lhsT
                            nc.tensor.matmul(
                                ps[3][:, ts(c, 256)],
                                l8,
                                w8ot[:, 2 * ks : 2 * ks + 2,
                                     c * 256 : (c + 1) * 256],
                                start=(ks == 0 and c == 0),
                                stop=False,
                                perf_mode=DR,
                                skip_group_check=True,
                            )
                for k in range(KO):
                    lb = xbs[:, k, ts(j, P)]
                    nc.tensor.matmul(
                        ps[2], lb, wgt[:, k, :], start=(k == 0), stop=(k == KO - 1)
                    )
                    if k >= 4:
                        nc.tensor.matmul(
                            ps[3], lb, wot[:, k, :],
                            start=False, stop=(k == KO - 1),
                            skip_group_check=True,
                        )

            def epilogue(ps, ct, rows, uid, splits=1):
                # psum order: 0=i 1=f 2=g 3=o. activation scale undoes the
                # host-side 16x W premultiply.
                if with_bias:
                    zs = []
                    for nn in range(NG):
                        z = gpool.tile([P, H], F32, tag=f"z{nn}", name=f"z{nn}_{uid}")
                        nc.vector.tensor_add(z[:], ps[nn], bias_t[:, ts(nn, H)])
                        zs.append(z)
                else:
                    zs = ps
                w_ = H // splits
                for q in range(splits):
                    cs = slice(q * w_, (q + 1) * w_)
                    # all four activations first: a gate's PSUM bank frees at
                    # its activation read, so front-loading them unblocks the
                    # bank rotation for btile N+2 ~2us earlier.
                    i_t = gpool.tile([P, w_], F32, tag="i_t", name=f"i_{uid}_{q}")
                    nc.scalar.activation(i_t[:], zs[0][:, cs], AF.Sigmoid, scale=SCL)
                    f_t = gpool.tile([P, w_], F32, tag="f_t", name=f"f_{uid}_{q}")
                    nc.scalar.activation(f_t[:], zs[1][:, cs], AF.Sigmoid, scale=SCL)
                    g_t = gpool.tile([P, w_], F32, tag="g_t", name=f"g_{uid}_{q}")
                    nc.scalar.activation(g_t[:], zs[2][:, cs], AF.Tanh, scale=SCL)
                    o_t = gpool.tile([P, w_], F32, tag="o_t", name=f"o_{uid}_{q}")
                    nc.scalar.activation(o_t[:], zs[3][:, cs], AF.Sigmoid, scale=SCL)

                    fc = gpool.tile([P, w_], F32, tag="fc", name=f"fc_{uid}_{q}")
                    nc.vector.tensor_mul(fc[:], f_t[:], ct[:, cs])
                    ig = gpool.tile([P, w_], F32, tag="ig", name=f"ig_{uid}_{q}")
                    nc.vector.tensor_mul(ig[:], i_t[:], g_t[:])
                    cn = gpool.tile([P, w_], BF16, tag="cn", name=f"cn_{uid}_{q}")
                    nc.vector.tensor_add(cn[:], fc[:], ig[:])
                    tn = gpool.tile([P, w_], F32, tag="tn", name=f"tn_{uid}_{q}")
                    nc.scalar.activation(tn[:], cn[:], AF.Tanh)
                    hn = gpool.tile([P, w_], BF16, tag="hn", name=f"hn_{uid}_{q}")
                    nc.vector.tensor_mul(hn[:], o_t[:], tn[:])

                    nc.sync.dma_start(c_out[rows, cs], cn[:])
                    nc.sync.dma_start(h_out[rows, cs], hn[:])
                return cn

            # Slab 0: j-pairs; within a pair run the fp8 phase first, then g,
            # then o's bf16 tail — matching the DMA stream arrival order.
            xslabs = {}

            def prefetch_slab(g):
                # The dma triggers sit on the sequential sync queue behind the
                # preceding epilogue's out-DMA triggers, which wait on that
                # epilogue's results — a natural throttle that keeps prefetch
                # from starving the slab-0 startup stream.
                x8s = x8pool.tile([P, KO, BG_ROWS], F8, tag="x8s", name=f"x8s_{g}")
                nc.sync.dma_start(x8s[:], x8t[g])
                xbs = xbpool.tile([P, KO, BG_ROWS], BF16, tag="xbs", name=f"xbs_{g}")
                nc.sync.dma_start(xbs[:], xbt[g])
                xslabs[g] = (x8s, xbs)

            for jp in (0, 2):
                ps2 = {
                    (j, nn): ppool.tile([P, H], F32, tag="ps", name=f"ps0_{j}_{nn}")
                    for j in (jp, jp + 1)
                    for nn in range(NG)
                }
                # phases interleaved with stream-half arrival order:
                # fp8 ks0-1 -> g k0-3 -> fp8 ks2-3 -> g k4-7 -> o bf16
                schedule = (
                    [("f8", 0), ("f8", 1)]
                    + [("g", k) for k in range(4)]
                    + [("f8", 2), ("f8", 3)]
                    + [("g", k) for k in range(4, KO)]
                )
                for kind, ks in schedule:
                    if kind == "g":
                        k = ks
                        for j in (jp, jp + 1):
                            nc.tensor.matmul(
                                ps2[(j, 2)], xbs0[:, k, ts(j, P)], wgt[:, k, :],
                                start=(k == 0), stop=(k == KO - 1),
                            )
                        continue
                    for j in (jp, jp + 1):
                        l8 = x8s0[:, 2 * ks : 2 * ks + 2, ts(j, P)]
                        for gi in (0, 1):
                            for c in range(2):
                                nc.tensor.matmul(
                                    ps2[(j, gi)][:, ts(c, 256)],
                                    l8,
                                    w8t[:, 2 * ks : 2 * ks + 2,
                                        gi * H + c * 256 : gi * H + (c + 1) * 256],
                                    start=(ks == 0 and c == 0),
                                    stop=(ks == KS - 1),
                                    perf_mode=DR,
                                    skip_group_check=True,
                                )
                        if ks < 2:
                            for c in range(2):
                                nc.tensor.matmul(
                                    ps2[(j, 3)][:, ts(c, 256)],
                                    l8,
                                    w8ot[:, 2 * ks : 2 * ks + 2,
                                         c * 256 : (c + 1) * 256],
                                    start=(ks == 0 and c == 0),
                                    stop=False,
                                    perf_mode=DR,
                                    skip_group_check=True,
                                )
                # gate o bf16 tail
                for k in range(4, KO):
                    for j in (jp, jp + 1):
                        nc.tensor.matmul(
                            ps2[(j, 3)], xbs0[:, k, ts(j, P)], wot[:, k, :],
                            start=False, stop=(k == KO - 1),
                            skip_group_check=True,
                        )
                for j in (jp, jp + 1):
                    epilogue(
                        [ps2[(j, nn)] for nn in range(NG)],
                        cts0[j],
                        slice(j * P, (j + 1) * P),
                        f"g0_{j}",
                    )
                    if j == 1:
                        prefetch_slab(1)
                    elif j == 3:
                        prefetch_slab(2)

            # Slabs 1..7 against prefetched slabs (next slab requested after
            # this slab's first epilogue — ~3 btiles of lead time).  Cell
            # tiles load one btile ahead: issuing the DMA before the current
            # epilogue keeps its trigger clear of the out-DMA triggers
            # (which block the sync queue on epilogue results).
            ct_next = cpool.tile([P, H], BF16, tag="ct", name="ct_4")
            nc.sync.dma_start(ct_next[:], cell[4 * P : 5 * P, :])
            for g in range(1, BG):
                x8s, xbs = xslabs[g]
                for j in range(BT_PER_BG):
                    bt = g * BT_PER_BG + j
                    rows = slice(bt * P, (bt + 1) * P)
                    ct = ct_next
                    ps = [
                        ppool.tile([P, H], F32, tag="ps", name=f"ps{nn}_{bt}")
                        for nn in range(NG)
                    ]
                    mm_btile(ps, x8s, xbs, j)
                    if bt + 1 < BG * BT_PER_BG:
                        ct_next = cpool.tile(
                            [P, H], BF16, tag="ct", name=f"ct_{bt + 1}"
                        )
                        nc.sync.dma_start(
                            ct_next[:], cell[(bt + 1) * P : (bt + 2) * P, :]
                        )
                    last_bt = g == BG - 1 and j == BT_PER_BG - 1
                    epilogue(ps, ct, rows, f"g{g}_{j}", splits=2 if last_bt else 1)
                    if j == 0 and g + 1 < BG:
                        prefetch_slab(g + 1)

    _split_multi_waits(nc)
    return nc


def kernel(input, cell_state, Wi, bi, Wf, bf, Wg, bg, Wo, bo):
    global LAST_RESULTS

    x = np.asarray(input, dtype=np.float32)
    cell = np.ascontiguousarray(np.asarray(cell_state, dtype=np.float32).astype(NPBF))
    Wi, Wf, Wg, Wo = (np.asarray(m, dtype=np.float32) for m in (Wi, Wf, Wg, Wo))
    bcat = np.concatenate(
        [np.asarray(v, dtype=np.float32) for v in (bi, bf, bg, bo)]
    )  # [4H] in i,f,g,o order (matches psum order)
    with_bias = bool(np.any(bcat))

    def wlay(Wcat, np_dt):
        # [D, N] -> [p, ko, n], scaled by WSCALE
        n = Wcat.shape[1]
        return np.ascontiguousarray(
            (Wcat * WSCALE).astype(np_dt).reshape(KO, P, n).transpose(1, 0, 2)
        )

    w8_dev = wlay(np.concatenate([Wi, Wf], axis=1), NPF8)          # [P,KO,1024]
    wb_dev = np.ascontiguousarray(
        np.stack([wlay(Wg, NPBF), wlay(Wo, NPBF)])
    )  # [2,P,KO,H]
    w8o_dev = np.ascontiguousarray(
        (Wo[:512] * WSCALE).astype(NPF8).reshape(4, P, H).transpose(1, 0, 2)
    )  # [P,4,H]

    in_maps = []
    for c in range(N_CORES):
        xc = x[c * B_LOC : (c + 1) * B_LOC]  # [4096, 1024]
        def xlay(np_dt):
            return np.ascontiguousarray(
                xc.astype(np_dt)
                .reshape(BG, BG_ROWS, KO, P)
                .transpose(0, 3, 2, 1)
            )
        m = {
            "x8t": xlay(NPF8),
            "xbt": xlay(NPBF),
            "w8": w8_dev,
            "w8o": w8o_dev,
            "wb": wb_dev,
            "cell": cell[c * B_LOC : (c + 1) * B_LOC],
        }
        if with_bias:
            m["bias"] = np.ascontiguousarray(
                np.broadcast_to(bcat[None, :] * WSCALE, (P, NW)).astype(np.float32)
            )
        in_maps.append(m)

    key = with_bias
    if key not in _CACHED:
        _CACHED[key] = _build(with_bias)
    nc = _CACHED[key]

    trace = os.environ.get("KERNEL_TRACE", "0") == "1"
    res = run_bass_kernel_spmd(nc, in_maps, list(range(N_CORES)), trace=trace)
    LAST_RESULTS = res

    h = np.concatenate(
        [res.results[c]["h_out"].astype(np.float32) for c in range(N_CORES)], axis=0
    )
    c_ = np.concatenate(
        [res.results[c]["c_out"].astype(np.float32) for c in range(N_CORES)], axis=0
    )
    return h, c_


# revision 8
# speedup vs baseline: 1.1040x; 1.0035x over previous
"""CustomLSTM cell on 8 TRN2 NeuronCores — fp8/bf16 hybrid matmul.

Data-parallel over batch (4096 rows/core). The 4 gate projections run at
mixed precision chosen so the total error stays ~1.5e-2 (< 2e-2 gate):
  - i, f gates: full-K fp8e4m3 DoubleRow matmuls (2x PE rate)
  - o gate:     fp8 DoubleRow on K 0:256, bf16 on K 256:1024
  - g (tanh) gate: full bf16 (its error coefficient is 3.6x the others)
All W variants are premultiplied by 16 on the host (fp8 subnormal
avoidance); the activation instructions fold the 1/16 dequant into their
scale operand. PSUM chains mix fp8/bf16 matmuls at a consistent scale.

PE work: 9/16 of the GEMM at 2x rate -> ~157us/core vs 218us all-bf16.

Self-contained: shapes/sharding hardcoded for
input [32768, 1024], cell_state [32768, 512], W* [1024, 512].
"""

import os

import numpy as np
import ml_dtypes

import bass_rust
import concourse.bass as bass
import concourse.mybir as mybir
import concourse.tile as tile
from concourse.bass_utils import run_bass_kernel_spmd

N_CORES = 8
B = 32768
D = 1024
H = 512
P = 128
B_LOC = B // N_CORES        # 4096 rows per core
KO = D // P                 # 8 k-subtiles of 128
KS = KO // 2                # 4 DoubleRow k-steps of 256
NW = 4 * H                  # 2048 concatenated gate dim
NG = 4
BG_ROWS = 512               # batch rows per x slab
BG = B_LOC // BG_ROWS       # 8 slabs per core
BT_PER_BG = BG_ROWS // P    # 4 batch tiles per slab

WSCALE = 16.0               # host premultiplies all W; undone in activation

F8 = mybir.dt.float8e4
BF16 = mybir.dt.bfloat16
F32 = mybir.dt.float32
NPF8 = ml_dtypes.float8_e4m3
NPBF = ml_dtypes.bfloat16

LAST_RESULTS = None
_CACHED = {}


def _split_multi_waits(nc):
    """Legalize for a walrus build that accepts one sync-wait per instruction."""
    n = 0
    for f in nc.m.functions:
        for blk in f.blocks:
            insts = blk.instructions
            if not any(
                i.sync_info is not None and len(i.sync_info.on_wait) > 1
                for i in insts
            ):
                continue
            out = []
            for inst in insts:
                si = inst.sync_info
                if si is not None and len(si.on_wait) > 1:
                    waits = list(si.on_wait)
                    for w in waits[:-1]:
                        nop = mybir.InstNoOp(name=f"waitsplit_{n}", ins=[], outs=[])
                        n += 1
                        nop.engine = inst.engine
                        nop.sync_info = bass_rust.SyncInfo(on_wait=[w], on_update=[])
                        out.append(nop)
                    inst.sync_info = bass_rust.SyncInfo(
                        on_wait=[waits[-1]], on_update=list(si.on_update)
                    )
                out.append(inst)
            blk.instructions = out


class _FastTailTileContext(tile.TileContext):
    """Drop both tail all-engine barriers.

    The stock tail is [drain+waits][barrier][sem/queue reset][barrier].  The
    drain's sem waits already cover completion of every instruction and DMA,
    so by the time the gpsimd-side reset runs nothing is in flight that could
    observe the cleared semaphores; NRT waits for each engine stream to halt
    independently.  Saves ~8-10us of EVSEM barrier ring.
    """

    def _drain_and_barrier(self, tick_clock, wait_clock):
        # No global-clock sem waits on the drain: every out-DMA trigger on
        # the sync queue already waits for its producer and precedes the
        # drain, so all engine work is transitively complete when the drain
        # runs; the drain itself waits for the DMA rings to empty.
        drain_inst = self.nc.sync.drain()
        # Chain the gpsimd-side reset directly behind the drain (gpsimd has
        # been idle since the warmup memset; without this it would clear
        # live semaphores immediately).
        tail_sem = self.nc.alloc_semaphore("fast_tail_sem")
        drain_inst.then_inc(tail_sem)
        self.nc.gpsimd.wait_ge(tail_sem, 1)
        assert self.sems is not None
        popped = self.nc._tile_sem_poison_stack.pop()
        assert popped is self._sem_poison
        self.nc.clear_and_free_semaphores(list(self.sems.allocated().values()))


def _build(with_bias):
    nc = bass.Bass()
    AF = mybir.ActivationFunctionType
    ts = bass.ts
    DR = mybir.MatmulPerfMode.DoubleRow
    SCL = 1.0 / WSCALE

    x8t = nc.dram_tensor("x8t", [BG, P, KO, BG_ROWS], F8, kind="ExternalInput")
    xbt = nc.dram_tensor("xbt", [BG, P, KO, BG_ROWS], BF16, kind="ExternalInput")
    w8 = nc.dram_tensor("w8", [P, KO, 2 * H], F8, kind="ExternalInput")   # i|f
    w8o = nc.dram_tensor("w8o", [P, 6, H], F8, kind="ExternalInput")      # o k<768
    wb = nc.dram_tensor("wb", [2, P, KO, H], BF16, kind="ExternalInput")  # [g,o]
    cell = nc.dram_tensor("cell", [B_LOC, H], BF16, kind="ExternalInput")
    if with_bias:
        bias = nc.dram_tensor("bias", [P, NW], F32, kind="ExternalInput")
    h_out = nc.dram_tensor("h_out", [B_LOC, H], BF16, kind="ExternalOutput")
    c_out = nc.dram_tensor("c_out", [B_LOC, H], BF16, kind="ExternalOutput")

    with _FastTailTileContext(nc) as tc:
        with (
            tc.tile_pool(name="wpool", bufs=1) as wpool,
            tc.tile_pool(name="x8pool", bufs=3) as x8pool,
            tc.tile_pool(name="xbpool", bufs=3) as xbpool,
            tc.tile_pool(name="cpool", bufs=4) as cpool,
            tc.tile_pool(name="gpool", bufs=3) as gpool,
            tc.tile_pool(name="ppool", bufs=8, space="PSUM") as ppool,
        ):
            # PE warmup first: runs while the startup DMA triggers issue, so
            # the HAM clock gate opens before real data lands.
            wz = wpool.tile([P, P], F8, tag="wz", name="wz")
            nc.gpsimd.memset(wz[:], 0.0)
            warm_ps = ppool.tile([P, P], F32, tag="ps", name="warm_ps")
            for _ in range(28):
                nc.tensor.matmul(warm_ps[:], wz[:], wz[:], start=True, stop=True)

            bias_t = None
            if with_bias:
                bias_t = wpool.tile([P, NW], F32, tag="bias_t", name="bias_t")
                nc.sync.dma_start(bias_t[:], bias[:])

            # Resident weight tiles + slab-0 x, DMA'd in consumption order:
            # fp8 stream first (slab 0 runs its fp8 phase first), then the
            # bf16 stream, with the first pair's cell tiles slotted where the
            # epilogues need them.
            w8t = wpool.tile([P, KO, 2 * H], F8, tag="w8t", name="w8t")
            w8ot = wpool.tile([P, 6, H], F8, tag="w8ot", name="w8ot")
            wgt = wpool.tile([P, KO, H], BF16, tag="wgt", name="wgt")
            wot = wpool.tile([P, KO, H], BF16, tag="wot", name="wot")
            x8s0 = wpool.tile([P, KO, BG_ROWS], F8, tag="x8s0", name="x8s0")
            xbs0 = wpool.tile([P, KO, BG_ROWS], BF16, tag="xbs0", name="xbs0")
            cts0 = [
                cpool.tile([P, H], BF16, tag="ct", name=f"ct0_{j}")
                for j in range(BT_PER_BG)
            ]
            # Half-slab granularity: the dram layouts are k-contiguous per
            # partition, so each half is 128 large descriptors instead of
            # 4x128 small ones — the sync queue's descriptor generation was
            # pacing the startup stream.
            for h2 in range(2):
                ks_ = slice(4 * h2, 4 * h2 + 4)
                nc.sync.dma_start(x8s0[:, ks_, :], x8t[0, :, ks_, :])
                nc.sync.dma_start(w8t[:, ks_, :], w8[:, ks_, :])
                w8o_hi = 2 if h2 == 0 else 6
                nc.sync.dma_start(w8ot[:, 2 * h2 : w8o_hi, :],
                                  w8o[:, 2 * h2 : w8o_hi, :])
                nc.sync.dma_start(xbs0[:, ks_, :], xbt[0, :, ks_, :])
                nc.sync.dma_start(wgt[:, ks_, :], wb[0, :, ks_, :])
            for j in (0, 1):
                nc.sync.dma_start(cts0[j][:], cell[j * P : (j + 1) * P, :])
            nc.sync.dma_start(wot[:, 6:, :], wb[1, :, 6:, :])  # o k6..7
            for j in (2, 3):
                nc.sync.dma_start(cts0[j][:], cell[j * P : (j + 1) * P, :])

            def mm_btile(ps, x8s, xbs, j):
                """i,f full fp8; o fp8 K<512 + bf16 k4..7; g full bf16."""
                for ks in range(KS):
                    l8 = x8s[:, 2 * ks : 2 * ks + 2, ts(j, P)]
                    for gi in (0, 1):  # i then f
                        for c in range(2):
                            nc.tensor.matmul(
                                ps[gi][:, ts(c, 256)],
                                l8,
                                w8t[:, 2 * ks : 2 * ks + 2,
                                    gi * H + c * 256 : gi * H + (c + 1) * 256],
                                start=(ks == 0 and c == 0),
                                stop=(ks == KS - 1),
                                perf_mode=DR,
                                skip_group_check=True,
                            )
                    if ks < 3:
                        for c in range(2):  # o's fp8 steps ride i/f's lhsT
                            nc.tensor.matmul(
                                ps[3][:, ts(c, 256)],
                                l8,
                                w8ot[:, 2 * ks : 2 * ks + 2,
                                     c * 256 : (c + 1) * 256],
                                start=(ks == 0 and c == 0),
                                stop=False,
                                perf_mode=DR,
                                skip_group_check=True,
                            )
                for k in range(KO):
                    lb = xbs[:, k, ts(j, P)]
                    nc.tensor.matmul(
                        ps[2], lb, wgt[:, k, :], start=(k == 0), stop=(k == KO - 1)
                    )
                    if k >= 6:
                        nc.tensor.matmul(
                            ps[3], lb, wot[:, k, :],
                            start=False, stop=(k == KO - 1),
                            skip_group_check=True,
                        )

            def epilogue(ps, ct, rows, uid, splits=1):
                # psum order: 0=i 1=f 2=g 3=o. activation scale undoes the
                # host-side 16x W premultiply.
                if with_bias:
                    zs = []
                    for nn in range(NG):
                        z = gpool.tile([P, H], F32, tag=f"z{nn}", name=f"z{nn}_{uid}")
                        nc.vector.tensor_add(z[:], ps[nn], bias_t[:, ts(nn, H)])
                        zs.append(z)
                else:
                    zs = ps
                w_ = H // splits
                for q in range(splits):
                    cs = slice(q * w_, (q + 1) * w_)
                    # all four activations first: a gate's PSUM bank frees at
                    # its activation read, so front-loading them unblocks the
                    # bank rotation for btile N+2 ~2us earlier.
                    i_t = gpool.tile([P, w_], F32, tag="i_t", name=f"i_{uid}_{q}")
                    nc.scalar.activation(i_t[:], zs[0][:, cs], AF.Sigmoid, scale=SCL)
                    f_t = gpool.tile([P, w_], F32, tag="f_t", name=f"f_{uid}_{q}")
                    nc.scalar.activation(f_t[:], zs[1][:, cs], AF.Sigmoid, scale=SCL)
                    g_t = gpool.tile([P, w_], F32, tag="g_t", name=f"g_{uid}_{q}")
                    nc.scalar.activation(g_t[:], zs[2][:, cs], AF.Tanh, scale=SCL)
                    o_t = gpool.tile([P, w_], F32, tag="o_t", name=f"o_{uid}_{q}")
                    nc.scalar.activation(o_t[:], zs[3][:, cs], AF.Sigmoid, scale=SCL)

                    fc = gpool.tile([P, w_], F32, tag="fc", name=f"fc_{uid}_{q}")
                    nc.vector.tensor_mul(fc[:], f_t[:], ct[:, cs])
                    ig = gpool.tile([P, w_], F32, tag="ig", name=f"ig_{uid}_{q}")
                    nc.vector.tensor_mul(ig[:], i_t[:], g_t[:])
                    cn = gpool.tile([P, w_], BF16, tag="cn", name=f"cn_{uid}_{q}")
                    nc.vector.tensor_add(cn[:], fc[:], ig[:])
                    tn = gpool.tile([P, w_], F32, tag="tn", name=f"tn_{uid}_{q}")
                    nc.scalar.activation(tn[:], cn[:], AF.Tanh)
                    hn = gpool.tile([P, w_], BF16, tag="hn", name=f"hn_{uid}_{q}")
                    nc.vector.tensor_mul(hn[:], o_t[:], tn[:])

                    nc.sync.dma_start(c_out[rows, cs], cn[:])
                    nc.sync.dma_start(h_out[rows, cs], hn[:])
                return cn

            # Slab 0: j-pairs; within a pair run the fp8 phase first, then g,
            # then o's bf16 tail — matching the DMA stream arrival order.
            xslabs = {}

            def prefetch_slab(g):
                # The dma triggers sit on the sequential sync queue behind the
                # preceding epilogue's out-DMA triggers, which wait on that
                # epilogue's results — a natural throttle that keeps prefetch
                # from starving the slab-0 startup stream.
                x8s = x8pool.tile([P, KO, BG_ROWS], F8, tag="x8s", name=f"x8s_{g}")
                nc.sync.dma_start(x8s[:], x8t[g])
                xbs = xbpool.tile([P, KO, BG_ROWS], BF16, tag="xbs", name=f"xbs_{g}")
                nc.sync.dma_start(xbs[:], xbt[g])
                xslabs[g] = (x8s, xbs)

            for jp in (0, 2):
                ps2 = {
                    (j, nn): ppool.tile([P, H], F32, tag="ps", name=f"ps0_{j}_{nn}")
                    for j in (jp, jp + 1)
                    for nn in range(NG)
                }
                # phases interleaved with stream-half arrival order:
                # fp8 ks0-1 -> g k0-3 -> fp8 ks2-3 -> g k4-7 -> o bf16
                schedule = (
                    [("f8", 0), ("f8", 1)]
                    + [("g", k) for k in range(4)]
                    + [("f8", 2), ("f8", 3)]
                    + [("g", k) for k in range(4, KO)]
                )
                for kind, ks in schedule:
                    if kind == "g":
                        k = ks
                        for j in (jp, jp + 1):
                            nc.tensor.matmul(
                                ps2[(j, 2)], xbs0[:, k, ts(j, P)], wgt[:, k, :],
                                start=(k == 0), stop=(k == KO - 1),
                            )
                        continue
                    for j in (jp, jp + 1):
                        l8 = x8s0[:, 2 * ks : 2 * ks + 2, ts(j, P)]
                        for gi in (0, 1):
                            for c in range(2):
                                nc.tensor.matmul(
                                    ps2[(j, gi)][:, ts(c, 256)],
                                    l8,
                                    w8t[:, 2 * ks : 2 * ks + 2,
                                        gi * H + c * 256 : gi * H + (c + 1) * 256],
                                    start=(ks == 0 and c == 0),
                                    stop=(ks == KS - 1),
                                    perf_mode=DR,
                                    skip_group_check=True,
                                )
                        if ks < 3:
                            for c in range(2):
                                nc.tensor.matmul(
                                    ps2[(j, 3)][:, ts(c, 256)],
                                    l8,
                                    w8ot[:, 2 * ks : 2 * ks + 2,
                                         c * 256 : (c + 1) * 256],
                                    start=(ks == 0 and c == 0),
                                    stop=False,
                                    perf_mode=DR,
                                    skip_group_check=True,
                                )
                # gate o bf16 tail
                for k in range(6, KO):
                    for j in (jp, jp + 1):
                        nc.tensor.matmul(
                            ps2[(j, 3)], xbs0[:, k, ts(j, P)], wot[:, k, :],
                            start=False, stop=(k == KO - 1),
                            skip_group_check=True,
                        )
                for j in (jp, jp + 1):
                    epilogue(
                        [ps2[(j, nn)] for nn in range(NG)],
                        cts0[j],
                        slice(j * P, (j + 1) * P),
                        f"g0_{j}",
                    )
                    if j == 1:
                        prefetch_slab(1)
                    elif j == 3:
                        prefetch_slab(2)

            # Slabs 1..7 against prefetched slabs (next slab requested after
            # this slab's first epilogue — ~3 btiles of lead time).  Cell
            # tiles load one btile ahead: issuing the DMA before the current
            # epilogue keeps its trigger clear of the out-DMA triggers
            # (which block the sync queue on epilogue results).
            ct_next = cpool.tile([P, H], BF16, tag="ct", name="ct_4")
            nc.sync.dma_start(ct_next[:], cell[4 * P : 5 * P, :])
            for g in range(1, BG):
                x8s, xbs = xslabs[g]
                for j in range(BT_PER_BG):
                    bt = g * BT_PER_BG + j
                    rows = slice(bt * P, (bt + 1) * P)
                    ct = ct_next
                    ps = [
                        ppool.tile([P, H], F32, tag="ps", name=f"ps{nn}_{bt}")
                        for nn in range(NG)
                    ]
                    mm_btile(ps, x8s, xbs, j)
                    if bt + 1 < BG * BT_PER_BG:
                        ct_next = cpool.tile(
                            [P, H], BF16, tag="ct", name=f"ct_{bt + 1}"
                        )
                        nc.sync.dma_start(
                            ct_next[:], cell[(bt + 1) * P : (bt + 2) * P, :]
                        )
                    last_bt = g == BG - 1 and j == BT_PER_BG - 1
                    epilogue(ps, ct, rows, f"g{g}_{j}", splits=2 if last_bt else 1)
                    if j == 0 and g + 1 < BG:
                        prefetch_slab(g + 1)

    _split_multi_waits(nc)
    return nc


def kernel(input, cell_state, Wi, bi, Wf, bf, Wg, bg, Wo, bo):
    global LAST_RESULTS

    x = np.asarray(input, dtype=np.float32)
    cell = np.ascontiguousarray(np.asarray(cell_state, dtype=np.float32).astype(NPBF))
    Wi, Wf, Wg, Wo = (np.asarray(m, dtype=np.float32) for m in (Wi, Wf, Wg, Wo))
    bcat = np.concatenate(
        [np.asarray(v, dtype=np.float32) for v in (bi, bf, bg, bo)]
    )  # [4H] in i,f,g,o order (matches psum order)
    with_bias = bool(np.any(bcat))

    def wlay(Wcat, np_dt):
        # [D, N] -> [p, ko, n], scaled by WSCALE
        n = Wcat.shape[1]
        return np.ascontiguousarray(
            (Wcat * WSCALE).astype(np_dt).reshape(KO, P, n).transpose(1, 0, 2)
        )

    w8_dev = wlay(np.concatenate([Wi, Wf], axis=1), NPF8)          # [P,KO,1024]
    wb_dev = np.ascontiguousarray(
        np.stack([wlay(Wg, NPBF), wlay(Wo, NPBF)])
    )  # [2,P,KO,H]
    w8o_dev = np.ascontiguousarray(
        (Wo[:768] * WSCALE).astype(NPF8).reshape(6, P, H).transpose(1, 0, 2)
    )  # [P,6,H]

    in_maps = []
    for c in range(N_CORES):
        xc = x[c * B_LOC : (c + 1) * B_LOC]  # [4096, 1024]
        def xlay(np_dt):
            return np.ascontiguousarray(
                xc.astype(np_dt)
                .reshape(BG, BG_ROWS, KO, P)
                .transpose(0, 3, 2, 1)
            )
        m = {
            "x8t": xlay(NPF8),
            "xbt": xlay(NPBF),
            "w8": w8_dev,
            "w8o": w8o_dev,
            "wb": wb_dev,
            "cell": cell[c * B_LOC : (c + 1) * B_LOC],
        }
        if with_bias:
            m["bias"] = np.ascontiguousarray(
                np.broadcast_to(bcat[None, :] * WSCALE, (P, NW)).astype(np.float32)
            )
        in_maps.append(m)

    key = with_bias
    if key not in _CACHED:
        _CACHED[key] = _build(with_bias)
    nc = _CACHED[key]

    trace = os.environ.get("KERNEL_TRACE", "0") == "1"
    res = run_bass_kernel_spmd(nc, in_maps, list(range(N_CORES)), trace=trace)
    LAST_RESULTS = res

    h = np.concatenate(
        [res.results[c]["h_out"].astype(np.float32) for c in range(N_CORES)], axis=0
    )
    c_ = np.concatenate(
        [res.results[c]["c_out"].astype(np.float32) for c in range(N_CORES)], axis=0
    )
    return h, c_
